# revision 11
# baseline (speedup 1.0000x reference)
"""Trainium2 Bass kernel for pooled cross-attention block (dense_transformer).

Reference computation per batch element b (B=8, one per NeuronCore):
  x2p = 2x2 mean-pool(x2)                      [512, 32, 32]
  Q = Wq @ x1  + bq                            [64, 4096]   (d-part layout)
  K = Wk @ x2p + bk                            [64, 1024]
  V = Wv @ x2p + bv                            [64, 1024]
  attn = softmax_n(Q^T K)                      [4096, 1024]
  out  = attn @ V^T                            [4096, 64]
  y    = out @ Wo^T + bo -> [256, 4096] ; result = x1 + y

Kernel strategy (all on-chip per core, streamed over n in 1024-col chunks):
  - scores computed TRANSPOSED: sT[m, n] = K^T Q so softmax's reduce dim m
    is the partition dim; the row-sum r[n] comes free from a ones column in
    the augmented V (row 64 of U = V_aug^T expS).
  - bias algebra (exact): bk drops (softmax shift invariance); bq enters via
    an extra K row (K_aug row 64 = bq^T K, paired with a ones row in Q_aug);
    bv folds into bo' = bo + Wo@bv (attn rows sum to 1); bo' enters via the
    ones-row of normalized U against an augmented Wo^T.
  - 2x2 pooling: two strided DVE adds; 1/4 folded into Wk/Wv on host.
  - performance: the PE must stream continuously to hold its warm 2.4 GHz
    clock, so the scores->exp->U loop is pipelined 2 deep (2-bank score
    tiles, 1024-wide exp), all DMAs are >=1MiB and loads are issued up
    front on sync (stores go via gpsimd SWDGE), and the reciprocal uses the
    fast custom-DVE approximation.
"""

import sys

for _p in ("/opt/trn_rl_repo",):
    if _p not in sys.path:
        sys.path.insert(0, _p)

import numpy as np

B, C1, C2, H, W, D = 8, 256, 512, 64, 64, 64
HW = H * W               # n (query) size: 4096
M = (H // 2) * (W // 2)  # kv size: 1024
NCH = 1024               # n-chunk
NCHUNKS = HW // NCH      # 4
C1T = C1 // 128          # 2
C2T = C2 // 128          # 4
MT = M // 128            # 8

# packed bf16 const layout (free-dim offsets)
_WK_OFF = 0              # [128, C2T, 65]
_WV_OFF = C2T * 65       # [128, C2T, 64]
_WO_OFF = _WV_OFF + C2T * 64   # [65->128, 256]
_CB_F = _WO_OFF + C1     # 772

_CACHE = {}


def _build():
    import concourse.bass as bass
    import concourse.tile as tile
    from concourse import bacc, mybir

    dt = mybir.dt
    f32, bf16, f32r = dt.float32, dt.bfloat16, dt.float32r
    Exp = mybir.ActivationFunctionType.Exp

    nc = bacc.Bacc(
        "TRN2", target_bir_lowering=False, debug=False, num_devices=8
    )
    x1 = nc.dram_tensor("x1", [C1, HW], f32r, kind="ExternalInput").ap()
    x2 = nc.dram_tensor("x2", [C2, HW], f32, kind="ExternalInput").ap()
    cb = nc.dram_tensor("cb", [128, _CB_F], bf16, kind="ExternalInput").ap()
    wq = nc.dram_tensor("wq", [128, C1T, D], f32r, kind="ExternalInput").ap()
    out = nc.dram_tensor("out", [C1, HW], f32, kind="ExternalOutput").ap()

    x1v = x1.rearrange("(t p) n -> p t n", t=C1T)
    x2v = x2.rearrange("(c p) n -> p c n", c=C2T)
    outv = out.rearrange("(t p) n -> p t n", t=C1T)

    from contextlib import ExitStack

    with tile.TileContext(nc) as tc, ExitStack() as ctx:
        pool = lambda name, bufs, **kw: ctx.enter_context(
            tc.tile_pool(name=name, bufs=bufs, **kw)
        )
        consts = pool("consts", 1)
        x2p = pool("x2p", 8)
        poolp = pool("poolp", 2)
        x1p = pool("x1p", 4)
        esp = pool("esp", 3)
        rp = pool("rp", 2)
        usp = pool("usp", 2)
        rbp = pool("rbp", 2)
        onp = pool("onp", 3)
        yop = pool("yop", 2)
        ps_s = pool("ps_s", 3, space="PSUM")   # [128,2,512] f32: 2 banks x3
        ps_u = pool("ps_u", 1, space="PSUM")   # [65,2,512] f32: 2 banks

        # ---- constants (2 DMAs) ---------------------------------------
        cb_sb = consts.tile([128, _CB_F], bf16, tag="cb")
        nc.sync.dma_start(out=cb_sb, in_=cb)
        wq_sb = consts.tile([128, C1T, D], f32r, tag="wq")
        nc.sync.dma_start(out=wq_sb, in_=wq)
        wk_v = cb_sb[:, _WK_OFF:_WK_OFF + C2T * 65].rearrange(
            "p (c d) -> p c d", c=C2T)
        wv_v = cb_sb[:, _WV_OFF:_WV_OFF + C2T * 64].rearrange(
            "p (c d) -> p c d", c=C2T)
        wo_v = cb_sb[0:D + 1, _WO_OFF:_WO_OFF + C1]

        # persistent activations
        s_bf = consts.tile([128, C2T, M], bf16, tag="sbf")     # pooled x2
        k_aug = consts.tile([D + 1, M], bf16, tag="kaug")
        v_aug = consts.tile([128, MT, D + 1], bf16, tag="vaug")
        q_aug = consts.tile([D + 1, HW], bf16, tag="qaug")
        nc.gpsimd.memset(q_aug[D:D + 1, :], 1.0)

        # ---- input DMAs (issue order == program order on sync) --------
        x2t = {}
        for h in range(2):
            for ci in range(C2T):
                t = x2p.tile([128, HW // 2], f32, tag="x2t", name=f"x2_{ci}_{h}")
                nc.sync.dma_start(
                    out=t, in_=x2v[:, ci, h * (HW // 2):(h + 1) * (HW // 2)]
                )
                x2t[(ci, h)] = t
            if h == 0:
                x1t0 = x1p.tile([128, C1T, NCH], f32r, tag="x1t", name="x1_0")
                nc.sync.dma_start(out=x1t0, in_=x1v[:, :, 0:NCH])
        x1t = [x1t0]
        for nj in range(1, NCHUNKS):
            x1t.append(
                x1p.tile([128, C1T, NCH], f32r, tag="x1t", name=f"x1_{nj}"))

        def load_x1(nj):
            nc.gpsimd.dma_start(
                out=x1t[nj], in_=x1v[:, :, nj * NCH:(nj + 1) * NCH])

        load_x1(1)

        # ---- phase A: pool x2, project K_aug and V^T ------------------
        k_ps = ps_u.tile([D + 1, 2, NCH // 2], f32, tag="psu", name="k_ps")
        v_ps = ps_s.tile([128, MT, D], f32, tag="pss", name="v_ps")
        for h in range(2):
            for ci in range(C2T):
                blk = x2t[(ci, h)].rearrange(
                    "p (hh w2 two) -> p hh w2 two", w2=W // 2, two=2)
                t1 = poolp.tile([128, (H // 2) * (W // 2)], f32,
                                tag="t1", name=f"t1_{ci}_{h}")
                t1v = t1.rearrange("p (hh w2) -> p hh w2", w2=W // 2)
                eng1 = nc.gpsimd if (h * C2T + ci) % 2 == 0 else nc.vector
                eng1.tensor_add(t1v, blk[:, :, :, 0], blk[:, :, :, 1])
                t2 = t1.rearrange("p (h2 two w2) -> p h2 two w2",
                                  two=2, w2=W // 2)
                sdst = s_bf[:, ci, h * (M // 2):(h + 1) * (M // 2)].rearrange(
                    "p (h2 w2) -> p h2 w2", w2=W // 2)
                nc.vector.tensor_add(sdst, t2[:, :, 0, :], t2[:, :, 1, :])
                nc.tensor.matmul(
                    k_ps[:, h, :],
                    lhsT=wk_v[:, ci, :],
                    rhs=s_bf[:, ci, h * (M // 2):(h + 1) * (M // 2)],
                    start=(ci == 0),
                    stop=(ci == C2T - 1),
                )
            # V^T accumulation: ci must be the INNER loop — start=True
            # clears has_written for the whole PSUM bank, so a later mi's
            # start would wipe an in-flight mi's accumulation bits.
            for mj in range(MT // 2):
                mi = h * (MT // 2) + mj
                for ci in range(C2T):
                    nc.tensor.matmul(
                        v_ps[:, mi, :],
                        lhsT=s_bf[:, ci, mi * 128:(mi + 1) * 128],
                        rhs=wv_v[:, ci, :],
                        start=(ci == 0),
                        stop=(ci == C2T - 1),
                    )
        Copy = mybir.ActivationFunctionType.Copy
        for h in range(2):
            nc.scalar.activation(
                k_aug[:, h * (M // 2):(h + 1) * (M // 2)], k_ps[:, h, :], Copy)
        nc.gpsimd.memset(v_aug[:, :, D], 1.0)
        for mi in range(MT):
            nc.scalar.activation(v_aug[:, mi, 0:D], v_ps[:, mi, :], Copy)

        # ---- Q projections: nj=0,1 up front, nj+2 prefetched per chunk -
        def make_q(nj):
            q_ps = ps_s.tile([D, 2, NCH // 2], f32, tag="pss", name=f"q_ps{nj}")
            for hh in range(2):
                for ci in range(C1T):
                    nc.tensor.matmul(
                        q_ps[:, hh, :],
                        lhsT=wq_sb[:, ci, :],
                        rhs=x1t[nj][:, ci, hh * 512:(hh + 1) * 512],
                        start=(ci == 0),
                        stop=(ci == C1T - 1),
                    )
            nc.vector.tensor_copy(
                q_aug[0:D, nj * NCH:(nj + 1) * NCH],
                q_ps.rearrange("p h n -> p (h n)"))

        make_q(0)
        make_q(1)

        # ---- phase B: stream n-chunks ---------------------------------
        for nj in range(NCHUNKS):
            u_ps = ps_u.tile([D + 1, 2, 512], f32, tag="psu", name=f"u{nj}")
            for mi in range(MT):
                if mi == 2 and nj + 2 < NCHUNKS:
                    load_x1(nj + 2)
                if mi == 4 and nj + 2 < NCHUNKS:
                    make_q(nj + 2)
                s_t = ps_s.tile([128, 2, 512], f32, tag="pss", name=f"s{nj}_{mi}")
                for hh in range(2):
                    nc.tensor.matmul(
                        s_t[:, hh, :],
                        lhsT=k_aug[:, mi * 128:(mi + 1) * 128],
                        rhs=q_aug[:, nj * NCH + hh * 512:nj * NCH + (hh + 1) * 512],
                        start=True,
                        stop=True,
                    )
                es = esp.tile([128, 2, 512], bf16, tag="es", name=f"es{nj}_{mi}")
                for hh in range(2):
                    nc.scalar.activation(es[:, hh, :], s_t[:, hh, :], Exp)
                    nc.tensor.matmul(
                        u_ps[:, hh, :],
                        lhsT=v_aug[:, mi, :],
                        rhs=es[:, hh, :],
                        start=(mi == 0),
                        stop=(mi == MT - 1),
                    )
            # Deferred normalization: y_unnorm = Wo_aug @ U runs straight
            # from a bf16 copy of U (frees the single psu slot fast), and the
            # 1/r scale is applied per column on the y tiles afterwards.
            # The row sum is staged to SBUF via ACT (reciprocal_approx_fast
            # misbehaves on PSUM inputs).
            rsb = rp.tile([1, 2, 512], f32, tag="rsb", name=f"rs{nj}")
            nc.scalar.activation(
                rsb.rearrange("p h n -> p (h n)"),
                u_ps[D:D + 1, :, :].rearrange("p h n -> p (h n)"), Copy)
            u_bf = usp.tile([D + 1, NCH], bf16, tag="usb", name=f"ub{nj}")
            nc.vector.tensor_copy(u_bf, u_ps.rearrange("p h n -> p (h n)"))
            for hh in range(2):
                rin = rp.tile([1, 512], f32, tag="rin", name=f"ri{nj}_{hh}")
                nc.vector.reciprocal_approx_fast(rin, rsb[:, hh, :])
                rb = rbp.tile([128, 512], f32, tag="rb", name=f"rb{nj}_{hh}")
                nc.gpsimd.partition_broadcast(rb, rin)
                yo = yop.tile([128, C1T, 512], f32, tag="yo",
                              name=f"yo{nj}_{hh}")
                for t in range(C1T):
                    y_ps = ps_s.tile([128, 512], f32, tag="pss",
                                     name=f"y{nj}_{hh}_{t}")
                    nc.tensor.matmul(
                        y_ps,
                        lhsT=wo_v[:, t * 128:(t + 1) * 128],
                        rhs=u_bf[:, hh * 512:(hh + 1) * 512],
                        start=True,
                        stop=True,
                    )
                    ys = onp.tile([128, 512], f32, tag="ys",
                                  name=f"ys{nj}_{hh}_{t}")
                    nc.vector.tensor_mul(ys, y_ps, rb)
                    nc.vector.tensor_add(
                        yo[:, t, :],
                        x1t[nj][:, t, hh * 512:(hh + 1) * 512].bitcast(f32),
                        ys,
                    )
                nc.gpsimd.dma_start(
                    out=outv[:, :, nj * NCH + hh * 512:nj * NCH + (hh + 1) * 512],
                    in_=yo)
    nc.compile()
    return nc


def _get_nc():
    if "nc" not in _CACHE:
        _CACHE["nc"] = _build()
    return _CACHE["nc"]


def _prep_in_maps(x1, x2, Wq, bq, Wk, bk, Wv, bv, Wo, bo):
    import ml_dtypes

    bf16 = ml_dtypes.bfloat16
    f32 = np.float32
    x1 = np.asarray(x1, f32)
    x2 = np.asarray(x2, f32)
    Wq = np.asarray(Wq, f32)
    Wk = np.asarray(Wk, f32)
    Wv = np.asarray(Wv, f32)
    Wo = np.asarray(Wo, f32)
    bq = np.asarray(bq, f32)
    bk = np.asarray(bk, f32)
    bv = np.asarray(bv, f32)
    bo = np.asarray(bo, f32)

    # bk is softmax-invariant (constant per score column) and is dropped.
    # bq enters scores via K_aug row 64 = bq^T K (paired with Q ones row).
    # bv folds into the output bias because attention rows sum to one.
    wk = 0.25 * Wk
    wk_aug = np.concatenate([wk, (bq @ wk)[None, :]], axis=0)    # [65, C2]
    wk_t = np.ascontiguousarray(
        wk_aug.T.reshape(C2T, 128, D + 1).transpose(1, 0, 2).reshape(128, -1)
    )  # [128, C2T*65]
    wv_t = np.ascontiguousarray(
        (0.25 * Wv).T.reshape(C2T, 128, D).transpose(1, 0, 2).reshape(128, -1)
    )  # [128, C2T*64]
    bo_eff = bo + Wo @ bv
    wo_aug = np.concatenate([Wo.T, bo_eff[None, :]], axis=0)     # [65, C1]
    wo_pad = np.zeros((128, C1), f32)
    wo_pad[: D + 1] = wo_aug
    cbuf = np.concatenate([wk_t, wv_t, wo_pad], axis=1).astype(bf16)
    assert cbuf.shape == (128, _CB_F)

    wqt = np.ascontiguousarray(
        Wq.T.reshape(C1T, 128, D).transpose(1, 0, 2)
    )  # [128, C1T, D]

    shared = {"cb": cbuf, "wq": wqt}
    in_maps = []
    for b in range(B):
        m = dict(shared)
        m["x1"] = np.ascontiguousarray(x1[b].reshape(C1, HW))
        m["x2"] = np.ascontiguousarray(x2[b].reshape(C2, HW))
        in_maps.append(m)
    return in_maps


def run(inputs, trace=False, **trace_kwargs):
    from concourse.bass_utils import run_bass_kernel_spmd

    nc = _get_nc()
    in_maps = _prep_in_maps(**inputs)
    res = run_bass_kernel_spmd(
        nc, in_maps, list(range(B)), trace=trace, **trace_kwargs
    )
    out = np.stack([res.results[i]["out"] for i in range(B)])
    out = out.reshape(B, C1, H, W).astype(np.float32)
    return out, res


def kernel(**inputs) -> np.ndarray:
    out, _ = run(inputs, trace=False)
    return out


# revision 12
# speedup vs baseline: 1.0381x; 1.0381x over previous
"""Trainium2 Bass kernel for pooled cross-attention block (dense_transformer).

Reference computation per batch element b (B=8, one per NeuronCore):
  x2p = 2x2 mean-pool(x2)                      [512, 32, 32]
  Q = Wq @ x1  + bq                            [64, 4096]   (d-part layout)
  K = Wk @ x2p + bk                            [64, 1024]
  V = Wv @ x2p + bv                            [64, 1024]
  attn = softmax_n(Q^T K)                      [4096, 1024]
  out  = attn @ V^T                            [4096, 64]
  y    = out @ Wo^T + bo -> [256, 4096] ; result = x1 + y

Kernel strategy (all on-chip per core, streamed over n in 1024-col chunks):
  - scores computed TRANSPOSED: sT[m, n] = K^T Q so softmax's reduce dim m
    is the partition dim; the row-sum r[n] comes free from a ones column in
    the augmented V (row 64 of U = V_aug^T expS).
  - bias algebra (exact): bk drops (softmax shift invariance); bq enters via
    an extra K row (K_aug row 64 = bq^T K, paired with a ones row in Q_aug);
    bv folds into bo' = bo + Wo@bv (attn rows sum to 1); bo' enters via the
    ones-row of normalized U against an augmented Wo^T.
  - 2x2 pooling: two strided DVE adds; 1/4 folded into Wk/Wv on host.
  - performance: the PE must stream continuously to hold its warm 2.4 GHz
    clock, so the scores->exp->U loop is pipelined 2 deep (2-bank score
    tiles, 1024-wide exp), all DMAs are >=1MiB and loads are issued up
    front on sync (stores go via gpsimd SWDGE), and the reciprocal uses the
    fast custom-DVE approximation.
"""

import sys

for _p in ("/opt/trn_rl_repo",):
    if _p not in sys.path:
        sys.path.insert(0, _p)

import numpy as np

B, C1, C2, H, W, D = 8, 256, 512, 64, 64, 64
HW = H * W               # n (query) size: 4096
M = (H // 2) * (W // 2)  # kv size: 1024
NCH = 1024               # n-chunk
NCHUNKS = HW // NCH      # 4
C1T = C1 // 128          # 2
C2T = C2 // 128          # 4
MT = M // 128            # 8

# packed bf16 const layout (free-dim offsets)
_WK_OFF = 0              # [128, C2T, 65]
_WV_OFF = C2T * 65       # [128, C2T, 64]
_WO_OFF = _WV_OFF + C2T * 64   # [65->128, 256]
_CB_F = _WO_OFF + C1     # 772

_CACHE = {}


def _build():
    import concourse.bass as bass
    import concourse.tile as tile
    from concourse import bacc, mybir

    dt = mybir.dt
    f32, bf16, f32r = dt.float32, dt.bfloat16, dt.float32r
    Exp = mybir.ActivationFunctionType.Exp

    nc = bacc.Bacc(
        "TRN2", target_bir_lowering=False, debug=False, num_devices=8
    )
    x1 = nc.dram_tensor("x1", [C1, HW], f32r, kind="ExternalInput").ap()
    x2 = nc.dram_tensor("x2", [C2, HW], f32, kind="ExternalInput").ap()
    cb = nc.dram_tensor("cb", [128, _CB_F], bf16, kind="ExternalInput").ap()
    wq = nc.dram_tensor("wq", [128, C1T, D], f32r, kind="ExternalInput").ap()
    out = nc.dram_tensor("out", [C1, HW], f32, kind="ExternalOutput").ap()

    x1v = x1.rearrange("(t p) n -> p t n", t=C1T)
    x2v = x2.rearrange("(c p) n -> p c n", c=C2T)
    outv = out.rearrange("(t p) n -> p t n", t=C1T)

    from contextlib import ExitStack

    with tile.TileContext(nc) as tc, ExitStack() as ctx:
        pool = lambda name, bufs, **kw: ctx.enter_context(
            tc.tile_pool(name=name, bufs=bufs, **kw)
        )
        consts = pool("consts", 1)
        x2p = pool("x2p", 8)
        poolp = pool("poolp", 2)
        x1p = pool("x1p", 4)
        esp = pool("esp", 3)
        rp = pool("rp", 2)
        usp = pool("usp", 2)
        rbp = pool("rbp", 2)
        onp = pool("onp", 3)
        yop = pool("yop", 2)
        ps_s = pool("ps_s", 3, space="PSUM")   # [128,2,512] f32: 2 banks x3
        ps_u = pool("ps_u", 1, space="PSUM")   # [65,2,512] f32: 2 banks

        # ---- constants (2 DMAs) ---------------------------------------
        cb_sb = consts.tile([128, _CB_F], bf16, tag="cb")
        nc.sync.dma_start(out=cb_sb, in_=cb)
        wq_sb = consts.tile([128, C1T, D], f32r, tag="wq")
        nc.sync.dma_start(out=wq_sb, in_=wq)
        wk_v = cb_sb[:, _WK_OFF:_WK_OFF + C2T * 65].rearrange(
            "p (c d) -> p c d", c=C2T)
        wv_v = cb_sb[:, _WV_OFF:_WV_OFF + C2T * 64].rearrange(
            "p (c d) -> p c d", c=C2T)
        wo_v = cb_sb[0:D + 1, _WO_OFF:_WO_OFF + C1]

        # persistent activations
        s_bf = consts.tile([128, C2T, M], bf16, tag="sbf")     # pooled x2
        k_aug = consts.tile([D + 1, M], bf16, tag="kaug")
        v_aug = consts.tile([128, MT, D + 1], bf16, tag="vaug")
        q_aug = consts.tile([D + 1, HW], bf16, tag="qaug")
        nc.gpsimd.memset(q_aug[D:D + 1, :], 1.0)

        # ---- input DMAs (issue order == program order on sync) --------
        x2t = {}
        for h in range(2):
            for ci in range(C2T):
                t = x2p.tile([128, HW // 2], f32, tag="x2t", name=f"x2_{ci}_{h}")
                nc.sync.dma_start(
                    out=t, in_=x2v[:, ci, h * (HW // 2):(h + 1) * (HW // 2)]
                )
                x2t[(ci, h)] = t
            if h == 0:
                x1t0 = x1p.tile([128, C1T, NCH], f32r, tag="x1t", name="x1_0")
                nc.sync.dma_start(out=x1t0, in_=x1v[:, :, 0:NCH])
        x1t = [x1t0]
        for nj in range(1, NCHUNKS):
            x1t.append(
                x1p.tile([128, C1T, NCH], f32r, tag="x1t", name=f"x1_{nj}"))

        def load_x1(nj):
            nc.gpsimd.dma_start(
                out=x1t[nj], in_=x1v[:, :, nj * NCH:(nj + 1) * NCH])

        load_x1(1)

        # ---- phase A: pool x2, project K_aug and V^T ------------------
        k_ps = ps_u.tile([D + 1, 2, NCH // 2], f32, tag="psu", name="k_ps")
        v_ps = ps_s.tile([128, MT, D], f32, tag="pss", name="v_ps")
        for h in range(2):
            for ci in range(C2T):
                blk = x2t[(ci, h)].rearrange(
                    "p (hh w2 two) -> p hh w2 two", w2=W // 2, two=2)
                t1 = poolp.tile([128, (H // 2) * (W // 2)], f32,
                                tag="t1", name=f"t1_{ci}_{h}")
                t1v = t1.rearrange("p (hh w2) -> p hh w2", w2=W // 2)
                eng1 = nc.gpsimd if (h * C2T + ci) % 2 == 0 else nc.vector
                eng1.tensor_add(t1v, blk[:, :, :, 0], blk[:, :, :, 1])
                t2 = t1.rearrange("p (h2 two w2) -> p h2 two w2",
                                  two=2, w2=W // 2)
                sdst = s_bf[:, ci, h * (M // 2):(h + 1) * (M // 2)].rearrange(
                    "p (h2 w2) -> p h2 w2", w2=W // 2)
                nc.vector.tensor_add(sdst, t2[:, :, 0, :], t2[:, :, 1, :])
                nc.tensor.matmul(
                    k_ps[:, h, :],
                    lhsT=wk_v[:, ci, :],
                    rhs=s_bf[:, ci, h * (M // 2):(h + 1) * (M // 2)],
                    start=(ci == 0),
                    stop=(ci == C2T - 1),
                )
            # V^T accumulation: ci must be the INNER loop — start=True
            # clears has_written for the whole PSUM bank, so a later mi's
            # start would wipe an in-flight mi's accumulation bits.
            for mj in range(MT // 2):
                mi = h * (MT // 2) + mj
                for ci in range(C2T):
                    nc.tensor.matmul(
                        v_ps[:, mi, :],
                        lhsT=s_bf[:, ci, mi * 128:(mi + 1) * 128],
                        rhs=wv_v[:, ci, :],
                        start=(ci == 0),
                        stop=(ci == C2T - 1),
                    )
        Copy = mybir.ActivationFunctionType.Copy
        for h in range(2):
            nc.scalar.activation(
                k_aug[:, h * (M // 2):(h + 1) * (M // 2)], k_ps[:, h, :], Copy)
        nc.gpsimd.memset(v_aug[:, :, D], 1.0)
        for mi in range(MT):
            nc.scalar.activation(v_aug[:, mi, 0:D], v_ps[:, mi, :], Copy)

        # ---- Q projections: nj=0,1 up front, nj+2 prefetched per chunk -
        def make_q(nj):
            q_ps = ps_s.tile([D, 2, NCH // 2], f32, tag="pss", name=f"q_ps{nj}")
            for hh in range(2):
                for ci in range(C1T):
                    nc.tensor.matmul(
                        q_ps[:, hh, :],
                        lhsT=wq_sb[:, ci, :],
                        rhs=x1t[nj][:, ci, hh * 512:(hh + 1) * 512],
                        start=(ci == 0),
                        stop=(ci == C1T - 1),
                    )
            nc.vector.tensor_copy(
                q_aug[0:D, nj * NCH:(nj + 1) * NCH],
                q_ps.rearrange("p h n -> p (h n)"))

        make_q(0)
        make_q(1)

        # ---- phase B: stream n-chunks ---------------------------------
        for nj in range(NCHUNKS):
            u_ps = ps_u.tile([D + 1, 2, 512], f32, tag="psu", name=f"u{nj}")
            for mi in range(MT):
                if mi == 2 and nj + 2 < NCHUNKS:
                    load_x1(nj + 2)
                if mi == 4 and nj + 2 < NCHUNKS:
                    make_q(nj + 2)
                s_t = ps_s.tile([128, 2, 512], f32, tag="pss", name=f"s{nj}_{mi}")
                for hh in range(2):
                    nc.tensor.matmul(
                        s_t[:, hh, :],
                        lhsT=k_aug[:, mi * 128:(mi + 1) * 128],
                        rhs=q_aug[:, nj * NCH + hh * 512:nj * NCH + (hh + 1) * 512],
                        start=True,
                        stop=True,
                    )
                es = esp.tile([128, 2, 512], bf16, tag="es", name=f"es{nj}_{mi}")
                nc.scalar.activation(
                    es.rearrange("p h n -> p (h n)"),
                    s_t.rearrange("p h n -> p (h n)"),
                    Exp,
                )
                for hh in range(2):
                    nc.tensor.matmul(
                        u_ps[:, hh, :],
                        lhsT=v_aug[:, mi, :],
                        rhs=es[:, hh, :],
                        start=(mi == 0),
                        stop=(mi == MT - 1),
                    )
            # Deferred normalization: y_unnorm = Wo_aug @ U runs straight
            # from a bf16 copy of U (frees the single psu slot fast), and the
            # 1/r scale is applied per column on the y tiles afterwards.
            # The row sum is staged to SBUF via ACT (reciprocal_approx_fast
            # misbehaves on PSUM inputs).
            rsb = rp.tile([1, 2, 512], f32, tag="rsb", name=f"rs{nj}")
            nc.scalar.activation(
                rsb.rearrange("p h n -> p (h n)"),
                u_ps[D:D + 1, :, :].rearrange("p h n -> p (h n)"), Copy)
            u_bf = usp.tile([D + 1, NCH], bf16, tag="usb", name=f"ub{nj}")
            nc.vector.tensor_copy(u_bf, u_ps.rearrange("p h n -> p (h n)"))
            for hh in range(2):
                rin = rp.tile([1, 512], f32, tag="rin", name=f"ri{nj}_{hh}")
                nc.vector.reciprocal_approx_fast(rin, rsb[:, hh, :])
                rb = rbp.tile([128, 512], f32, tag="rb", name=f"rb{nj}_{hh}")
                nc.gpsimd.partition_broadcast(rb, rin)
                yo = yop.tile([128, C1T, 512], f32, tag="yo",
                              name=f"yo{nj}_{hh}")
                for t in range(C1T):
                    y_ps = ps_s.tile([128, 512], f32, tag="pss",
                                     name=f"y{nj}_{hh}_{t}")
                    nc.tensor.matmul(
                        y_ps,
                        lhsT=wo_v[:, t * 128:(t + 1) * 128],
                        rhs=u_bf[:, hh * 512:(hh + 1) * 512],
                        start=True,
                        stop=True,
                    )
                    ys = onp.tile([128, 512], f32, tag="ys",
                                  name=f"ys{nj}_{hh}_{t}")
                    nc.vector.tensor_mul(ys, y_ps, rb)
                    nc.vector.tensor_add(
                        yo[:, t, :],
                        x1t[nj][:, t, hh * 512:(hh + 1) * 512].bitcast(f32),
                        ys,
                    )
                nc.gpsimd.dma_start(
                    out=outv[:, :, nj * NCH + hh * 512:nj * NCH + (hh + 1) * 512],
                    in_=yo)
    nc.compile()
    return nc


def _get_nc():
    if "nc" not in _CACHE:
        _CACHE["nc"] = _build()
    return _CACHE["nc"]


def _prep_in_maps(x1, x2, Wq, bq, Wk, bk, Wv, bv, Wo, bo):
    import ml_dtypes

    bf16 = ml_dtypes.bfloat16
    f32 = np.float32
    x1 = np.asarray(x1, f32)
    x2 = np.asarray(x2, f32)
    Wq = np.asarray(Wq, f32)
    Wk = np.asarray(Wk, f32)
    Wv = np.asarray(Wv, f32)
    Wo = np.asarray(Wo, f32)
    bq = np.asarray(bq, f32)
    bk = np.asarray(bk, f32)
    bv = np.asarray(bv, f32)
    bo = np.asarray(bo, f32)

    # bk is softmax-invariant (constant per score column) and is dropped.
    # bq enters scores via K_aug row 64 = bq^T K (paired with Q ones row).
    # bv folds into the output bias because attention rows sum to one.
    wk = 0.25 * Wk
    wk_aug = np.concatenate([wk, (bq @ wk)[None, :]], axis=0)    # [65, C2]
    wk_t = np.ascontiguousarray(
        wk_aug.T.reshape(C2T, 128, D + 1).transpose(1, 0, 2).reshape(128, -1)
    )  # [128, C2T*65]
    wv_t = np.ascontiguousarray(
        (0.25 * Wv).T.reshape(C2T, 128, D).transpose(1, 0, 2).reshape(128, -1)
    )  # [128, C2T*64]
    bo_eff = bo + Wo @ bv
    wo_aug = np.concatenate([Wo.T, bo_eff[None, :]], axis=0)     # [65, C1]
    wo_pad = np.zeros((128, C1), f32)
    wo_pad[: D + 1] = wo_aug
    cbuf = np.concatenate([wk_t, wv_t, wo_pad], axis=1).astype(bf16)
    assert cbuf.shape == (128, _CB_F)

    wqt = np.ascontiguousarray(
        Wq.T.reshape(C1T, 128, D).transpose(1, 0, 2)
    )  # [128, C1T, D]

    shared = {"cb": cbuf, "wq": wqt}
    in_maps = []
    for b in range(B):
        m = dict(shared)
        m["x1"] = np.ascontiguousarray(x1[b].reshape(C1, HW))
        m["x2"] = np.ascontiguousarray(x2[b].reshape(C2, HW))
        in_maps.append(m)
    return in_maps


def run(inputs, trace=False, **trace_kwargs):
    from concourse.bass_utils import run_bass_kernel_spmd

    nc = _get_nc()
    in_maps = _prep_in_maps(**inputs)
    res = run_bass_kernel_spmd(
        nc, in_maps, list(range(B)), trace=trace, **trace_kwargs
    )
    out = np.stack([res.results[i]["out"] for i in range(B)])
    out = out.reshape(B, C1, H, W).astype(np.float32)
    return out, res


def kernel(**inputs) -> np.ndarray:
    out, _ = run(inputs, trace=False)
    return out


# revision 13
# speedup vs baseline: 1.1524x; 1.1101x over previous
"""Trainium2 Bass kernel for pooled cross-attention block (dense_transformer).

Reference computation per batch element b (B=8, one per NeuronCore):
  x2p = 2x2 mean-pool(x2)                      [512, 32, 32]
  Q = Wq @ x1  + bq                            [64, 4096]   (d-part layout)
  K = Wk @ x2p + bk                            [64, 1024]
  V = Wv @ x2p + bv                            [64, 1024]
  attn = softmax_n(Q^T K)                      [4096, 1024]
  out  = attn @ V^T                            [4096, 64]
  y    = out @ Wo^T + bo -> [256, 4096] ; result = x1 + y

Kernel strategy (all on-chip per core, streamed over n in 1024-col chunks):
  - scores computed TRANSPOSED: sT[m, n] = K^T Q so softmax's reduce dim m
    is the partition dim; the row-sum r[n] comes free from a ones column in
    the augmented V (row 64 of U = V_aug^T expS).
  - bias algebra (exact): bk drops (softmax shift invariance); bq enters via
    an extra K row (K_aug row 64 = bq^T K, paired with a ones row in Q_aug);
    bv folds into bo' = bo + Wo@bv (attn rows sum to 1); bo' enters via the
    ones-row of normalized U against an augmented Wo^T.
  - 2x2 pooling: two strided DVE adds; 1/4 folded into Wk/Wv on host.
  - performance: the PE must stream continuously to hold its warm 2.4 GHz
    clock, so the scores->exp->U loop is pipelined 2 deep (2-bank score
    tiles, 1024-wide exp), all DMAs are >=1MiB and loads are issued up
    front on sync (stores go via gpsimd SWDGE), and the reciprocal uses the
    fast custom-DVE approximation.
"""

import sys

for _p in ("/opt/trn_rl_repo",):
    if _p not in sys.path:
        sys.path.insert(0, _p)

import numpy as np

B, C1, C2, H, W, D = 8, 256, 512, 64, 64, 64
HW = H * W               # n (query) size: 4096
M = (H // 2) * (W // 2)  # kv size: 1024
NCH = 1024               # n-chunk
NCHUNKS = HW // NCH      # 4
C1T = C1 // 128          # 2
C2T = C2 // 128          # 4
MT = M // 128            # 8

# packed bf16 const layout (free-dim offsets)
_WK_OFF = 0              # [128, C2T, 65]
_WV_OFF = C2T * 65       # [128, C2T, 64]
_WO_OFF = _WV_OFF + C2T * 64   # [65->128, 256]
_CB_F = _WO_OFF + C1     # 772

_CACHE = {}


def _build():
    import concourse.bass as bass
    import concourse.tile as tile
    from concourse import bacc, mybir

    dt = mybir.dt
    f32, bf16, f32r = dt.float32, dt.bfloat16, dt.float32r
    Exp = mybir.ActivationFunctionType.Exp

    nc = bacc.Bacc(
        "TRN2", target_bir_lowering=False, debug=False, num_devices=8
    )
    x1 = nc.dram_tensor("x1", [C1, HW], f32r, kind="ExternalInput").ap()
    x2 = nc.dram_tensor("x2", [C2, HW], f32, kind="ExternalInput").ap()
    cb = nc.dram_tensor("cb", [128, _CB_F], bf16, kind="ExternalInput").ap()
    wq = nc.dram_tensor("wq", [128, C1T, D], f32r, kind="ExternalInput").ap()
    out = nc.dram_tensor("out", [C1, HW], f32, kind="ExternalOutput").ap()

    x1v = x1.rearrange("(t p) n -> p t n", t=C1T)
    x2v = x2.rearrange("(c p) n -> p c n", c=C2T)
    outv = out.rearrange("(t p) n -> p t n", t=C1T)

    from contextlib import ExitStack

    with tile.TileContext(nc) as tc, ExitStack() as ctx:
        pool = lambda name, bufs, **kw: ctx.enter_context(
            tc.tile_pool(name=name, bufs=bufs, **kw)
        )
        consts = pool("consts", 1)
        x2p = pool("x2p", 8)
        poolp = pool("poolp", 2)
        x1p = pool("x1p", 4)
        esp = pool("esp", 3)
        rp = pool("rp", 2)
        usp = pool("usp", 2)
        rbp = pool("rbp", 2)
        onp = pool("onp", 3)
        yop = pool("yop", 2)
        ps_s = pool("ps_s", 2, space="PSUM")   # [128,2,512] f32: 2 banks x2
        ps_u = pool("ps_u", 1, space="PSUM")   # [65,2,512] f32: 2 banks
        ps_y = pool("ps_y", 2, space="PSUM")   # [128,512] f32: 1 bank x2

        # ---- constants (2 DMAs) ---------------------------------------
        cb_sb = consts.tile([128, _CB_F], bf16, tag="cb")
        nc.sync.dma_start(out=cb_sb, in_=cb)
        wq_sb = consts.tile([128, C1T, D], f32r, tag="wq")
        nc.sync.dma_start(out=wq_sb, in_=wq)
        wk_v = cb_sb[:, _WK_OFF:_WK_OFF + C2T * 65].rearrange(
            "p (c d) -> p c d", c=C2T)
        wv_v = cb_sb[:, _WV_OFF:_WV_OFF + C2T * 64].rearrange(
            "p (c d) -> p c d", c=C2T)
        wo_v = cb_sb[0:D + 1, _WO_OFF:_WO_OFF + C1]

        # persistent activations
        s_bf = consts.tile([128, C2T, M], bf16, tag="sbf")     # pooled x2
        k_aug = consts.tile([D + 1, M], bf16, tag="kaug")
        v_aug = consts.tile([128, MT, D + 1], bf16, tag="vaug")
        q_aug = consts.tile([D + 1, HW], bf16, tag="qaug")
        nc.gpsimd.memset(q_aug[D:D + 1, :], 1.0)

        # ---- input DMAs (issue order == program order on sync) --------
        x2t = {}
        for h in range(2):
            for ci in range(C2T):
                t = x2p.tile([128, HW // 2], f32, tag="x2t", name=f"x2_{ci}_{h}")
                nc.sync.dma_start(
                    out=t, in_=x2v[:, ci, h * (HW // 2):(h + 1) * (HW // 2)]
                )
                x2t[(ci, h)] = t
            if h == 0:
                x1t0 = x1p.tile([128, C1T, NCH], f32r, tag="x1t", name="x1_0")
                nc.sync.dma_start(out=x1t0, in_=x1v[:, :, 0:NCH])
        x1t = [x1t0]
        for nj in range(1, NCHUNKS):
            x1t.append(
                x1p.tile([128, C1T, NCH], f32r, tag="x1t", name=f"x1_{nj}"))

        def load_x1(nj):
            nc.gpsimd.dma_start(
                out=x1t[nj], in_=x1v[:, :, nj * NCH:(nj + 1) * NCH])

        load_x1(1)

        # ---- phase A: pool x2, project K_aug and V^T ------------------
        k_ps = ps_u.tile([D + 1, 2, NCH // 2], f32, tag="psu", name="k_ps")
        v_ps = ps_y.tile([128, MT, D], f32, tag="psy", name="v_ps")
        for h in range(2):
            for ci in range(C2T):
                blk = x2t[(ci, h)].rearrange(
                    "p (hh w2 two) -> p hh w2 two", w2=W // 2, two=2)
                t1 = poolp.tile([128, (H // 2) * (W // 2)], f32,
                                tag="t1", name=f"t1_{ci}_{h}")
                t1v = t1.rearrange("p (hh w2) -> p hh w2", w2=W // 2)
                eng1 = nc.gpsimd if (h * C2T + ci) % 2 == 0 else nc.vector
                eng1.tensor_add(t1v, blk[:, :, :, 0], blk[:, :, :, 1])
                t2 = t1.rearrange("p (h2 two w2) -> p h2 two w2",
                                  two=2, w2=W // 2)
                sdst = s_bf[:, ci, h * (M // 2):(h + 1) * (M // 2)].rearrange(
                    "p (h2 w2) -> p h2 w2", w2=W // 2)
                nc.vector.tensor_add(sdst, t2[:, :, 0, :], t2[:, :, 1, :])
                nc.tensor.matmul(
                    k_ps[:, h, :],
                    lhsT=wk_v[:, ci, :],
                    rhs=s_bf[:, ci, h * (M // 2):(h + 1) * (M // 2)],
                    start=(ci == 0),
                    stop=(ci == C2T - 1),
                )
            # V^T accumulation: ci must be the INNER loop — start=True
            # clears has_written for the whole PSUM bank, so a later mi's
            # start would wipe an in-flight mi's accumulation bits.
            for mj in range(MT // 2):
                mi = h * (MT // 2) + mj
                for ci in range(C2T):
                    nc.tensor.matmul(
                        v_ps[:, mi, :],
                        lhsT=s_bf[:, ci, mi * 128:(mi + 1) * 128],
                        rhs=wv_v[:, ci, :],
                        start=(ci == 0),
                        stop=(ci == C2T - 1),
                    )
        Copy = mybir.ActivationFunctionType.Copy
        for h in range(2):
            nc.scalar.activation(
                k_aug[:, h * (M // 2):(h + 1) * (M // 2)], k_ps[:, h, :], Copy)
        nc.gpsimd.memset(v_aug[:, :, D], 1.0)
        for mi in range(MT):
            nc.scalar.activation(v_aug[:, mi, 0:D], v_ps[:, mi, :], Copy)

        # ---- Q projections: nj=0,1 up front, nj+2 prefetched per chunk -
        def make_q(nj):
            q_ps = ps_s.tile([D, 2, NCH // 2], f32, tag="pss", name=f"q_ps{nj}")
            for hh in range(2):
                for ci in range(C1T):
                    nc.tensor.matmul(
                        q_ps[:, hh, :],
                        lhsT=wq_sb[:, ci, :],
                        rhs=x1t[nj][:, ci, hh * 512:(hh + 1) * 512],
                        start=(ci == 0),
                        stop=(ci == C1T - 1),
                    )
            nc.vector.tensor_copy(
                q_aug[0:D, nj * NCH:(nj + 1) * NCH],
                q_ps.rearrange("p h n -> p (h n)"))

        make_q(0)
        make_q(1)

        # ---- phase B: stream n-chunks ---------------------------------
        for nj in range(NCHUNKS):
            u_ps = ps_u.tile([D + 1, 2, 512], f32, tag="psu", name=f"u{nj}")
            for mi in range(MT):
                if mi == 2 and nj + 2 < NCHUNKS:
                    load_x1(nj + 2)
                if mi == 4 and nj + 2 < NCHUNKS:
                    make_q(nj + 2)
                s_t = ps_s.tile([128, 2, 512], f32, tag="pss", name=f"s{nj}_{mi}")
                for hh in range(2):
                    nc.tensor.matmul(
                        s_t[:, hh, :],
                        lhsT=k_aug[:, mi * 128:(mi + 1) * 128],
                        rhs=q_aug[:, nj * NCH + hh * 512:nj * NCH + (hh + 1) * 512],
                        start=True,
                        stop=True,
                    )
                es = esp.tile([128, 2, 512], bf16, tag="es", name=f"es{nj}_{mi}")
                nc.scalar.activation(
                    es.rearrange("p h n -> p (h n)"),
                    s_t.rearrange("p h n -> p (h n)"),
                    Exp,
                )
                for hh in range(2):
                    nc.tensor.matmul(
                        u_ps[:, hh, :],
                        lhsT=v_aug[:, mi, :],
                        rhs=es[:, hh, :],
                        start=(mi == 0),
                        stop=(mi == MT - 1),
                    )
            # Deferred normalization: y_unnorm = Wo_aug @ U runs straight
            # from a bf16 copy of U (frees the single psu slot fast), and the
            # 1/r scale is applied per column on the y tiles afterwards.
            # The row sum is staged to SBUF via ACT (reciprocal_approx_fast
            # misbehaves on PSUM inputs).
            rsb = rp.tile([1, 2, 512], f32, tag="rsb", name=f"rs{nj}")
            nc.scalar.activation(
                rsb.rearrange("p h n -> p (h n)"),
                u_ps[D:D + 1, :, :].rearrange("p h n -> p (h n)"), Copy)
            u_bf = usp.tile([D + 1, NCH], bf16, tag="usb", name=f"ub{nj}")
            nc.vector.tensor_copy(u_bf, u_ps.rearrange("p h n -> p (h n)"))
            for hh in range(2):
                rin = rp.tile([1, 512], f32, tag="rin", name=f"ri{nj}_{hh}")
                nc.vector.reciprocal_approx_fast(rin, rsb[:, hh, :])
                rb = rbp.tile([128, 512], f32, tag="rb", name=f"rb{nj}_{hh}")
                nc.gpsimd.partition_broadcast(rb, rin)
                yo = yop.tile([128, C1T, 512], f32, tag="yo",
                              name=f"yo{nj}_{hh}")
                for t in range(C1T):
                    y_ps = ps_y.tile([128, 512], f32, tag="psy",
                                     name=f"y{nj}_{hh}_{t}")
                    nc.tensor.matmul(
                        y_ps,
                        lhsT=wo_v[:, t * 128:(t + 1) * 128],
                        rhs=u_bf[:, hh * 512:(hh + 1) * 512],
                        start=True,
                        stop=True,
                    )
                    ys = onp.tile([128, 512], f32, tag="ys",
                                  name=f"ys{nj}_{hh}_{t}")
                    nc.vector.tensor_mul(ys, y_ps, rb)
                    nc.vector.tensor_add(
                        yo[:, t, :],
                        x1t[nj][:, t, hh * 512:(hh + 1) * 512].bitcast(f32),
                        ys,
                    )
                nc.gpsimd.dma_start(
                    out=outv[:, :, nj * NCH + hh * 512:nj * NCH + (hh + 1) * 512],
                    in_=yo)
    nc.compile()
    return nc


def _get_nc():
    if "nc" not in _CACHE:
        _CACHE["nc"] = _build()
    return _CACHE["nc"]


def _prep_in_maps(x1, x2, Wq, bq, Wk, bk, Wv, bv, Wo, bo):
    import ml_dtypes

    bf16 = ml_dtypes.bfloat16
    f32 = np.float32
    x1 = np.asarray(x1, f32)
    x2 = np.asarray(x2, f32)
    Wq = np.asarray(Wq, f32)
    Wk = np.asarray(Wk, f32)
    Wv = np.asarray(Wv, f32)
    Wo = np.asarray(Wo, f32)
    bq = np.asarray(bq, f32)
    bk = np.asarray(bk, f32)
    bv = np.asarray(bv, f32)
    bo = np.asarray(bo, f32)

    # bk is softmax-invariant (constant per score column) and is dropped.
    # bq enters scores via K_aug row 64 = bq^T K (paired with Q ones row).
    # bv folds into the output bias because attention rows sum to one.
    wk = 0.25 * Wk
    wk_aug = np.concatenate([wk, (bq @ wk)[None, :]], axis=0)    # [65, C2]
    wk_t = np.ascontiguousarray(
        wk_aug.T.reshape(C2T, 128, D + 1).transpose(1, 0, 2).reshape(128, -1)
    )  # [128, C2T*65]
    wv_t = np.ascontiguousarray(
        (0.25 * Wv).T.reshape(C2T, 128, D).transpose(1, 0, 2).reshape(128, -1)
    )  # [128, C2T*64]
    bo_eff = bo + Wo @ bv
    wo_aug = np.concatenate([Wo.T, bo_eff[None, :]], axis=0)     # [65, C1]
    wo_pad = np.zeros((128, C1), f32)
    wo_pad[: D + 1] = wo_aug
    cbuf = np.concatenate([wk_t, wv_t, wo_pad], axis=1).astype(bf16)
    assert cbuf.shape == (128, _CB_F)

    wqt = np.ascontiguousarray(
        Wq.T.reshape(C1T, 128, D).transpose(1, 0, 2)
    )  # [128, C1T, D]

    shared = {"cb": cbuf, "wq": wqt}
    in_maps = []
    for b in range(B):
        m = dict(shared)
        m["x1"] = np.ascontiguousarray(x1[b].reshape(C1, HW))
        m["x2"] = np.ascontiguousarray(x2[b].reshape(C2, HW))
        in_maps.append(m)
    return in_maps


def run(inputs, trace=False, **trace_kwargs):
    from concourse.bass_utils import run_bass_kernel_spmd

    nc = _get_nc()
    in_maps = _prep_in_maps(**inputs)
    res = run_bass_kernel_spmd(
        nc, in_maps, list(range(B)), trace=trace, **trace_kwargs
    )
    out = np.stack([res.results[i]["out"] for i in range(B)])
    out = out.reshape(B, C1, H, W).astype(np.float32)
    return out, res


def kernel(**inputs) -> np.ndarray:
    out, _ = run(inputs, trace=False)
    return out


# revision 15
# speedup vs baseline: 1.2192x; 1.0579x over previous
"""Trainium2 Bass kernel for pooled cross-attention block (dense_transformer).

Reference computation per batch element b (B=8, one per NeuronCore):
  x2p = 2x2 mean-pool(x2)                      [512, 32, 32]
  Q = Wq @ x1  + bq                            [64, 4096]   (d-part layout)
  K = Wk @ x2p + bk                            [64, 1024]
  V = Wv @ x2p + bv                            [64, 1024]
  attn = softmax_n(Q^T K)                      [4096, 1024]
  out  = attn @ V^T                            [4096, 64]
  y    = out @ Wo^T + bo -> [256, 4096] ; result = x1 + y

Kernel strategy (all on-chip per core, streamed over n in 1024-col chunks):
  - scores computed TRANSPOSED: sT[m, n] = K^T Q so softmax's reduce dim m
    is the partition dim; the row-sum r[n] comes free from a ones column in
    the augmented V (row 64 of U = V_aug^T expS).
  - bias algebra (exact): bk drops (softmax shift invariance); bq enters via
    an extra K row (K_aug row 64 = bq^T K, paired with a ones row in Q_aug);
    bv folds into bo' = bo + Wo@bv (attn rows sum to 1); bo' enters via the
    ones-row of normalized U against an augmented Wo^T.
  - 2x2 pooling: two strided DVE adds; 1/4 folded into Wk/Wv on host.
  - performance: the PE must stream continuously to hold its warm 2.4 GHz
    clock, so the scores->exp->U loop is pipelined 2 deep (2-bank score
    tiles, 1024-wide exp), all DMAs are >=1MiB and loads are issued up
    front on sync (stores go via gpsimd SWDGE), and the reciprocal uses the
    fast custom-DVE approximation.
"""

import sys

for _p in ("/opt/trn_rl_repo",):
    if _p not in sys.path:
        sys.path.insert(0, _p)

import numpy as np

B, C1, C2, H, W, D = 8, 256, 512, 64, 64, 64
HW = H * W               # n (query) size: 4096
M = (H // 2) * (W // 2)  # kv size: 1024
NCH = 1024               # n-chunk
NCHUNKS = HW // NCH      # 4
C1T = C1 // 128          # 2
C2T = C2 // 128          # 4
MT = M // 128            # 8

# packed bf16 const layout (free-dim offsets)
_WK_OFF = 0              # [128, C2T, 65]
_WV_OFF = C2T * 65       # [128, C2T, 64]
_WO_OFF = _WV_OFF + C2T * 64   # [65->128, 256]
_CB_F = _WO_OFF + C1     # 772

_CACHE = {}


def _build():
    import concourse.bass as bass
    import concourse.tile as tile
    from concourse import bacc, mybir

    dt = mybir.dt
    f32, bf16, f32r = dt.float32, dt.bfloat16, dt.float32r
    Exp = mybir.ActivationFunctionType.Exp

    nc = bacc.Bacc(
        "TRN2", target_bir_lowering=False, debug=False, num_devices=8
    )
    x1 = nc.dram_tensor("x1", [C1, HW], f32r, kind="ExternalInput").ap()
    x2 = nc.dram_tensor("x2", [C2, HW], f32, kind="ExternalInput").ap()
    cb = nc.dram_tensor("cb", [128, _CB_F], bf16, kind="ExternalInput").ap()
    wq = nc.dram_tensor("wq", [128, C1T, D], f32r, kind="ExternalInput").ap()
    out = nc.dram_tensor("out", [C1, HW], f32, kind="ExternalOutput").ap()

    x1v = x1.rearrange("(t p) n -> p t n", t=C1T)
    x2v = x2.rearrange("(c p) n -> p c n", c=C2T)
    outv = out.rearrange("(t p) n -> p t n", t=C1T)

    from contextlib import ExitStack

    with tile.TileContext(nc) as tc, ExitStack() as ctx:
        pool = lambda name, bufs, **kw: ctx.enter_context(
            tc.tile_pool(name=name, bufs=bufs, **kw)
        )
        consts = pool("consts", 1)
        x2p = pool("x2p", 8)
        poolp = pool("poolp", 2)
        x1p = pool("x1p", 4)
        esp = pool("esp", 3)
        rp = pool("rp", 2)
        usp = pool("usp", 2)
        rbp = pool("rbp", 2)
        onp = pool("onp", 3)
        yop = pool("yop", 2)
        ps_s = pool("ps_s", 2, space="PSUM")   # [128,2,512] f32: 2 banks x2
        ps_u = pool("ps_u", 1, space="PSUM")   # [65,2,512] f32: 2 banks
        ps_y = pool("ps_y", 2, space="PSUM")   # [128,512] f32: 1 bank x2

        # ---- constants (2 DMAs) ---------------------------------------
        cb_sb = consts.tile([128, _CB_F], bf16, tag="cb")
        nc.sync.dma_start(out=cb_sb, in_=cb)
        wq_sb = consts.tile([128, C1T, D], f32r, tag="wq")
        nc.sync.dma_start(out=wq_sb, in_=wq)
        wk_v = cb_sb[:, _WK_OFF:_WK_OFF + C2T * 65].rearrange(
            "p (c d) -> p c d", c=C2T)
        wv_v = cb_sb[:, _WV_OFF:_WV_OFF + C2T * 64].rearrange(
            "p (c d) -> p c d", c=C2T)
        wo_v = cb_sb[0:D + 1, _WO_OFF:_WO_OFF + C1]

        # persistent activations
        s_bf = consts.tile([128, C2T, M], bf16, tag="sbf")     # pooled x2
        k_aug = consts.tile([D + 1, M], bf16, tag="kaug")
        v_aug = consts.tile([128, MT, D + 1], bf16, tag="vaug")
        q_aug = consts.tile([D + 1, HW], bf16, tag="qaug")
        nc.gpsimd.memset(q_aug[D:D + 1, :], 1.0)

        # ---- input DMAs (issue order == program order on sync) --------
        x2t = {}
        x1t = [x1p.tile([128, C1T, NCH], f32r, tag="x1t", name=f"x1_{nj}")
               for nj in range(NCHUNKS)]

        def load_x1(nj):
            nc.sync.dma_start(
                out=x1t[nj], in_=x1v[:, :, nj * NCH:(nj + 1) * NCH])

        for h in range(2):
            for ci in range(C2T):
                t = x2p.tile([128, HW // 2], f32, tag="x2t", name=f"x2_{ci}_{h}")
                nc.sync.dma_start(
                    out=t, in_=x2v[:, ci, h * (HW // 2):(h + 1) * (HW // 2)]
                )
                x2t[(ci, h)] = t
            if h == 0:
                load_x1(0)
                load_x1(1)
        load_x1(2)
        load_x1(3)

        # ---- phase A: pool x2, project K_aug and V^T ------------------
        k_ps = ps_u.tile([D + 1, 2, NCH // 2], f32, tag="psu", name="k_ps")
        v_ps = ps_y.tile([128, MT, D], f32, tag="psy", name="v_ps")
        for h in range(2):
            for ci in range(C2T):
                blk = x2t[(ci, h)].rearrange(
                    "p (hh w2 two) -> p hh w2 two", w2=W // 2, two=2)
                t1 = poolp.tile([128, (H // 2) * (W // 2)], f32,
                                tag="t1", name=f"t1_{ci}_{h}")
                t1v = t1.rearrange("p (hh w2) -> p hh w2", w2=W // 2)
                eng1 = nc.gpsimd if (h * C2T + ci) % 2 == 0 else nc.vector
                eng1.tensor_add(t1v, blk[:, :, :, 0], blk[:, :, :, 1])
                t2 = t1.rearrange("p (h2 two w2) -> p h2 two w2",
                                  two=2, w2=W // 2)
                sdst = s_bf[:, ci, h * (M // 2):(h + 1) * (M // 2)].rearrange(
                    "p (h2 w2) -> p h2 w2", w2=W // 2)
                nc.vector.tensor_add(sdst, t2[:, :, 0, :], t2[:, :, 1, :])
                nc.tensor.matmul(
                    k_ps[:, h, :],
                    lhsT=wk_v[:, ci, :],
                    rhs=s_bf[:, ci, h * (M // 2):(h + 1) * (M // 2)],
                    start=(ci == 0),
                    stop=(ci == C2T - 1),
                )
            # V^T accumulation: ci must be the INNER loop — start=True
            # clears has_written for the whole PSUM bank, so a later mi's
            # start would wipe an in-flight mi's accumulation bits.
            for mj in range(MT // 2):
                mi = h * (MT // 2) + mj
                for ci in range(C2T):
                    nc.tensor.matmul(
                        v_ps[:, mi, :],
                        lhsT=s_bf[:, ci, mi * 128:(mi + 1) * 128],
                        rhs=wv_v[:, ci, :],
                        start=(ci == 0),
                        stop=(ci == C2T - 1),
                    )
        Copy = mybir.ActivationFunctionType.Copy
        for h in range(2):
            nc.scalar.activation(
                k_aug[:, h * (M // 2):(h + 1) * (M // 2)], k_ps[:, h, :], Copy)
        nc.gpsimd.memset(v_aug[:, :, D], 1.0)
        for mi in range(MT):
            nc.scalar.activation(v_aug[:, mi, 0:D], v_ps[:, mi, :], Copy)

        # ---- Q projections: nj=0,1 up front, nj+2 prefetched per chunk -
        def make_q(nj):
            q_ps = ps_s.tile([D, 2, NCH // 2], f32, tag="pss", name=f"q_ps{nj}")
            for hh in range(2):
                for ci in range(C1T):
                    nc.tensor.matmul(
                        q_ps[:, hh, :],
                        lhsT=wq_sb[:, ci, :],
                        rhs=x1t[nj][:, ci, hh * 512:(hh + 1) * 512],
                        start=(ci == 0),
                        stop=(ci == C1T - 1),
                    )
            nc.vector.tensor_copy(
                q_aug[0:D, nj * NCH:(nj + 1) * NCH],
                q_ps.rearrange("p h n -> p (h n)"))

        make_q(0)
        make_q(1)

        # ---- phase B: stream n-chunks ---------------------------------
        for nj in range(NCHUNKS):
            u_ps = ps_u.tile([D + 1, 2, 512], f32, tag="psu", name=f"u{nj}")
            for mi in range(MT):
                if mi == 4 and nj + 2 < NCHUNKS:
                    make_q(nj + 2)
                s_t = ps_s.tile([128, 2, 512], f32, tag="pss", name=f"s{nj}_{mi}")
                for hh in range(2):
                    nc.tensor.matmul(
                        s_t[:, hh, :],
                        lhsT=k_aug[:, mi * 128:(mi + 1) * 128],
                        rhs=q_aug[:, nj * NCH + hh * 512:nj * NCH + (hh + 1) * 512],
                        start=True,
                        stop=True,
                    )
                es = esp.tile([128, 2, 512], bf16, tag="es", name=f"es{nj}_{mi}")
                nc.scalar.activation(
                    es.rearrange("p h n -> p (h n)"),
                    s_t.rearrange("p h n -> p (h n)"),
                    Exp,
                )
                for hh in range(2):
                    nc.tensor.matmul(
                        u_ps[:, hh, :],
                        lhsT=v_aug[:, mi, :],
                        rhs=es[:, hh, :],
                        start=(mi == 0),
                        stop=(mi == MT - 1),
                    )
            # Deferred normalization: y_unnorm = Wo_aug @ U runs straight
            # from a bf16 copy of U (frees the single psu slot fast), and the
            # 1/r scale is applied per column on the y tiles afterwards.
            # The row sum is staged to SBUF via ACT (reciprocal_approx_fast
            # misbehaves on PSUM inputs).
            rsb = rp.tile([1, 2, 512], f32, tag="rsb", name=f"rs{nj}")
            nc.scalar.activation(
                rsb.rearrange("p h n -> p (h n)"),
                u_ps[D:D + 1, :, :].rearrange("p h n -> p (h n)"), Copy)
            u_bf = usp.tile([D + 1, NCH], bf16, tag="usb", name=f"ub{nj}")
            nc.vector.tensor_copy(u_bf, u_ps.rearrange("p h n -> p (h n)"))
            for hh in range(2):
                rin = rp.tile([1, 512], f32, tag="rin", name=f"ri{nj}_{hh}")
                nc.vector.reciprocal_approx_fast(rin, rsb[:, hh, :])
                rb = rbp.tile([128, 512], f32, tag="rb", name=f"rb{nj}_{hh}")
                nc.gpsimd.partition_broadcast(rb, rin)
                yo = yop.tile([128, C1T, 512], f32, tag="yo",
                              name=f"yo{nj}_{hh}")
                for t in range(C1T):
                    y_ps = ps_y.tile([128, 512], f32, tag="psy",
                                     name=f"y{nj}_{hh}_{t}")
                    nc.tensor.matmul(
                        y_ps,
                        lhsT=wo_v[:, t * 128:(t + 1) * 128],
                        rhs=u_bf[:, hh * 512:(hh + 1) * 512],
                        start=True,
                        stop=True,
                    )
                    ys = onp.tile([128, 512], f32, tag="ys",
                                  name=f"ys{nj}_{hh}_{t}")
                    nc.vector.tensor_mul(ys, y_ps, rb)
                    nc.vector.tensor_add(
                        yo[:, t, :],
                        x1t[nj][:, t, hh * 512:(hh + 1) * 512].bitcast(f32),
                        ys,
                    )
                nc.gpsimd.dma_start(
                    out=outv[:, :, nj * NCH + hh * 512:nj * NCH + (hh + 1) * 512],
                    in_=yo)
    nc.compile()
    return nc


def _get_nc():
    if "nc" not in _CACHE:
        _CACHE["nc"] = _build()
    return _CACHE["nc"]


def _prep_in_maps(x1, x2, Wq, bq, Wk, bk, Wv, bv, Wo, bo):
    import ml_dtypes

    bf16 = ml_dtypes.bfloat16
    f32 = np.float32
    x1 = np.asarray(x1, f32)
    x2 = np.asarray(x2, f32)
    Wq = np.asarray(Wq, f32)
    Wk = np.asarray(Wk, f32)
    Wv = np.asarray(Wv, f32)
    Wo = np.asarray(Wo, f32)
    bq = np.asarray(bq, f32)
    bk = np.asarray(bk, f32)
    bv = np.asarray(bv, f32)
    bo = np.asarray(bo, f32)

    # bk is softmax-invariant (constant per score column) and is dropped.
    # bq enters scores via K_aug row 64 = bq^T K (paired with Q ones row).
    # bv folds into the output bias because attention rows sum to one.
    wk = 0.25 * Wk
    wk_aug = np.concatenate([wk, (bq @ wk)[None, :]], axis=0)    # [65, C2]
    wk_t = np.ascontiguousarray(
        wk_aug.T.reshape(C2T, 128, D + 1).transpose(1, 0, 2).reshape(128, -1)
    )  # [128, C2T*65]
    wv_t = np.ascontiguousarray(
        (0.25 * Wv).T.reshape(C2T, 128, D).transpose(1, 0, 2).reshape(128, -1)
    )  # [128, C2T*64]
    bo_eff = bo + Wo @ bv
    wo_aug = np.concatenate([Wo.T, bo_eff[None, :]], axis=0)     # [65, C1]
    wo_pad = np.zeros((128, C1), f32)
    wo_pad[: D + 1] = wo_aug
    cbuf = np.concatenate([wk_t, wv_t, wo_pad], axis=1).astype(bf16)
    assert cbuf.shape == (128, _CB_F)

    wqt = np.ascontiguousarray(
        Wq.T.reshape(C1T, 128, D).transpose(1, 0, 2)
    )  # [128, C1T, D]

    shared = {"cb": cbuf, "wq": wqt}
    in_maps = []
    for b in range(B):
        m = dict(shared)
        m["x1"] = np.ascontiguousarray(x1[b].reshape(C1, HW))
        m["x2"] = np.ascontiguousarray(x2[b].reshape(C2, HW))
        in_maps.append(m)
    return in_maps


def run(inputs, trace=False, **trace_kwargs):
    from concourse.bass_utils import run_bass_kernel_spmd

    nc = _get_nc()
    in_maps = _prep_in_maps(**inputs)
    res = run_bass_kernel_spmd(
        nc, in_maps, list(range(B)), trace=trace, **trace_kwargs
    )
    out = np.stack([res.results[i]["out"] for i in range(B)])
    out = out.reshape(B, C1, H, W).astype(np.float32)
    return out, res


def kernel(**inputs) -> np.ndarray:
    out, _ = run(inputs, trace=False)
    return out


# revision 16
# speedup vs baseline: 1.2248x; 1.0046x over previous
"""Trainium2 Bass kernel for pooled cross-attention block (dense_transformer).

Reference computation per batch element b (B=8, one per NeuronCore):
  x2p = 2x2 mean-pool(x2)                      [512, 32, 32]
  Q = Wq @ x1  + bq                            [64, 4096]   (d-part layout)
  K = Wk @ x2p + bk                            [64, 1024]
  V = Wv @ x2p + bv                            [64, 1024]
  attn = softmax_n(Q^T K)                      [4096, 1024]
  out  = attn @ V^T                            [4096, 64]
  y    = out @ Wo^T + bo -> [256, 4096] ; result = x1 + y

Kernel strategy (all on-chip per core, streamed over n in 1024-col chunks):
  - scores computed TRANSPOSED: sT[m, n] = K^T Q so softmax's reduce dim m
    is the partition dim; the row-sum r[n] comes free from a ones column in
    the augmented V (row 64 of U = V_aug^T expS).
  - bias algebra (exact): bk drops (softmax shift invariance); bq enters via
    an extra K row (K_aug row 64 = bq^T K, paired with a ones row in Q_aug);
    bv folds into bo' = bo + Wo@bv (attn rows sum to 1); bo' enters via the
    ones-row of normalized U against an augmented Wo^T.
  - 2x2 pooling: two strided DVE adds; 1/4 folded into Wk/Wv on host.
  - performance: the PE must stream continuously to hold its warm 2.4 GHz
    clock, so the scores->exp->U loop is pipelined 2 deep (2-bank score
    tiles, 1024-wide exp), all DMAs are >=1MiB and loads are issued up
    front on sync (stores go via gpsimd SWDGE), and the reciprocal uses the
    fast custom-DVE approximation.
"""

import sys

for _p in ("/opt/trn_rl_repo",):
    if _p not in sys.path:
        sys.path.insert(0, _p)

import numpy as np

B, C1, C2, H, W, D = 8, 256, 512, 64, 64, 64
HW = H * W               # n (query) size: 4096
M = (H // 2) * (W // 2)  # kv size: 1024
NCH = 1024               # n-chunk
NCHUNKS = HW // NCH      # 4
C1T = C1 // 128          # 2
C2T = C2 // 128          # 4
MT = M // 128            # 8

# packed bf16 const layout (free-dim offsets)
_WK_OFF = 0              # [128, C2T, 65]
_WV_OFF = C2T * 65       # [128, C2T, 64]
_WO_OFF = _WV_OFF + C2T * 64   # [65->128, 256]
_CB_F = _WO_OFF + C1     # 772

_CACHE = {}


def _build():
    import concourse.bass as bass
    import concourse.tile as tile
    from concourse import bacc, mybir

    dt = mybir.dt
    f32, bf16, f32r = dt.float32, dt.bfloat16, dt.float32r
    Exp = mybir.ActivationFunctionType.Exp

    nc = bacc.Bacc(
        "TRN2", target_bir_lowering=False, debug=False, num_devices=8
    )
    x1 = nc.dram_tensor("x1", [C1, HW], f32r, kind="ExternalInput").ap()
    x2 = nc.dram_tensor("x2", [C2, HW], f32, kind="ExternalInput").ap()
    cb = nc.dram_tensor("cb", [128, _CB_F], bf16, kind="ExternalInput").ap()
    wq = nc.dram_tensor("wq", [128, C1T, D], f32r, kind="ExternalInput").ap()
    out = nc.dram_tensor("out", [C1, HW], f32, kind="ExternalOutput").ap()

    x1v = x1.rearrange("(t p) n -> p t n", t=C1T)
    x2v = x2.rearrange("(c p) n -> p c n", c=C2T)
    outv = out.rearrange("(t p) n -> p t n", t=C1T)

    from contextlib import ExitStack

    with tile.TileContext(nc) as tc, ExitStack() as ctx:
        pool = lambda name, bufs, **kw: ctx.enter_context(
            tc.tile_pool(name=name, bufs=bufs, **kw)
        )
        consts = pool("consts", 1)
        x2p = pool("x2p", 8)
        poolp = pool("poolp", 2)
        x1p = pool("x1p", 4)
        esp = pool("esp", 3)
        rp = pool("rp", 2)
        usp = pool("usp", 2)
        rbp = pool("rbp", 2)
        onp = pool("onp", 3)
        yop = pool("yop", 2)
        ps_s = pool("ps_s", 2, space="PSUM")   # [128,2,512] f32: 2 banks x2
        ps_u = pool("ps_u", 1, space="PSUM")   # [65,2,512] f32: 2 banks
        ps_y = pool("ps_y", 2, space="PSUM")   # [128,512] f32: 1 bank x2

        # ---- constants (2 DMAs) ---------------------------------------
        cb_sb = consts.tile([128, _CB_F], bf16, tag="cb")
        nc.sync.dma_start(out=cb_sb, in_=cb)
        wq_sb = consts.tile([128, C1T, D], f32r, tag="wq")
        nc.sync.dma_start(out=wq_sb, in_=wq)
        wk_v = cb_sb[:, _WK_OFF:_WK_OFF + C2T * 65].rearrange(
            "p (c d) -> p c d", c=C2T)
        wv_v = cb_sb[:, _WV_OFF:_WV_OFF + C2T * 64].rearrange(
            "p (c d) -> p c d", c=C2T)
        wo_v = cb_sb[0:D + 1, _WO_OFF:_WO_OFF + C1]

        # persistent activations
        s_bf = consts.tile([128, C2T, M], bf16, tag="sbf")     # pooled x2
        k_aug = consts.tile([D + 1, M], bf16, tag="kaug")
        v_aug = consts.tile([128, MT, D + 1], bf16, tag="vaug")
        q_aug = consts.tile([D + 1, HW], bf16, tag="qaug")
        nc.gpsimd.memset(q_aug[D:D + 1, :], 1.0)

        # ---- input DMAs (issue order == program order on sync) --------
        x2t = {}
        x1t = [x1p.tile([128, C1T, NCH], f32r, tag="x1t", name=f"x1_{nj}")
               for nj in range(NCHUNKS)]

        def load_x1(nj):
            nc.sync.dma_start(
                out=x1t[nj], in_=x1v[:, :, nj * NCH:(nj + 1) * NCH])

        for h in range(2):
            for ci in range(C2T):
                t = x2p.tile([128, HW // 2], f32, tag="x2t", name=f"x2_{ci}_{h}")
                nc.sync.dma_start(
                    out=t, in_=x2v[:, ci, h * (HW // 2):(h + 1) * (HW // 2)]
                )
                x2t[(ci, h)] = t
            if h == 0:
                load_x1(0)
                load_x1(1)
        load_x1(2)
        load_x1(3)

        # ---- phase A: pool x2, project K_aug and V^T ------------------
        k_ps = ps_u.tile([D + 1, 2, NCH // 2], f32, tag="psu", name="k_ps")
        v_ps = ps_y.tile([128, MT, D], f32, tag="psy", name="v_ps")
        for h in range(2):
            for ci in range(C2T):
                blk = x2t[(ci, h)].rearrange(
                    "p (hh w2 two) -> p hh w2 two", w2=W // 2, two=2)
                t1 = poolp.tile([128, (H // 2) * (W // 2)], f32,
                                tag="t1", name=f"t1_{ci}_{h}")
                t1v = t1.rearrange("p (hh w2) -> p hh w2", w2=W // 2)
                eng1 = nc.gpsimd if (h * C2T + ci) % 2 == 0 else nc.vector
                eng1.tensor_add(t1v, blk[:, :, :, 0], blk[:, :, :, 1])
                t2 = t1.rearrange("p (h2 two w2) -> p h2 two w2",
                                  two=2, w2=W // 2)
                sdst = s_bf[:, ci, h * (M // 2):(h + 1) * (M // 2)].rearrange(
                    "p (h2 w2) -> p h2 w2", w2=W // 2)
                nc.vector.tensor_add(sdst, t2[:, :, 0, :], t2[:, :, 1, :])
                nc.tensor.matmul(
                    k_ps[:, h, :],
                    lhsT=wk_v[:, ci, :],
                    rhs=s_bf[:, ci, h * (M // 2):(h + 1) * (M // 2)],
                    start=(ci == 0),
                    stop=(ci == C2T - 1),
                )
            # V^T accumulation: ci must be the INNER loop — start=True
            # clears has_written for the whole PSUM bank, so a later mi's
            # start would wipe an in-flight mi's accumulation bits.
            for mj in range(MT // 2):
                mi = h * (MT // 2) + mj
                for ci in range(C2T):
                    nc.tensor.matmul(
                        v_ps[:, mi, :],
                        lhsT=s_bf[:, ci, mi * 128:(mi + 1) * 128],
                        rhs=wv_v[:, ci, :],
                        start=(ci == 0),
                        stop=(ci == C2T - 1),
                    )
            # evacuate this half's K and V^T immediately. h0 goes on ACT so
            # the first chunk's exps queue right behind it; h1 goes on DVE
            # to keep the late h1 work from head-of-line-blocking the exps.
            Copy = mybir.ActivationFunctionType.Copy
            if h == 0:
                nc.scalar.activation(
                    k_aug[:, 0:M // 2], k_ps[:, 0, :], Copy)
                for mi in range(MT // 2):
                    nc.scalar.activation(v_aug[:, mi, 0:D], v_ps[:, mi, :], Copy)
            else:
                nc.vector.tensor_copy(
                    k_aug[:, M // 2:M], k_ps[:, 1, :])
                for mi in range(MT // 2, MT):
                    nc.vector.tensor_copy(v_aug[:, mi, 0:D], v_ps[:, mi, :])
        nc.gpsimd.memset(v_aug[:, :, D], 1.0)

        # ---- Q projections: nj=0,1 up front, nj+2 prefetched per chunk -
        def make_q(nj):
            q_ps = ps_s.tile([D, 2, NCH // 2], f32, tag="pss", name=f"q_ps{nj}")
            for hh in range(2):
                for ci in range(C1T):
                    nc.tensor.matmul(
                        q_ps[:, hh, :],
                        lhsT=wq_sb[:, ci, :],
                        rhs=x1t[nj][:, ci, hh * 512:(hh + 1) * 512],
                        start=(ci == 0),
                        stop=(ci == C1T - 1),
                    )
            nc.vector.tensor_copy(
                q_aug[0:D, nj * NCH:(nj + 1) * NCH],
                q_ps.rearrange("p h n -> p (h n)"))

        make_q(0)
        make_q(1)

        # ---- phase B: stream n-chunks ---------------------------------
        for nj in range(NCHUNKS):
            u_ps = ps_u.tile([D + 1, 2, 512], f32, tag="psu", name=f"u{nj}")
            for mi in range(MT):
                if mi == 4 and nj + 2 < NCHUNKS:
                    make_q(nj + 2)
                s_t = ps_s.tile([128, 2, 512], f32, tag="pss", name=f"s{nj}_{mi}")
                for hh in range(2):
                    nc.tensor.matmul(
                        s_t[:, hh, :],
                        lhsT=k_aug[:, mi * 128:(mi + 1) * 128],
                        rhs=q_aug[:, nj * NCH + hh * 512:nj * NCH + (hh + 1) * 512],
                        start=True,
                        stop=True,
                    )
                es = esp.tile([128, 2, 512], bf16, tag="es", name=f"es{nj}_{mi}")
                nc.scalar.activation(
                    es.rearrange("p h n -> p (h n)"),
                    s_t.rearrange("p h n -> p (h n)"),
                    Exp,
                )
                for hh in range(2):
                    nc.tensor.matmul(
                        u_ps[:, hh, :],
                        lhsT=v_aug[:, mi, :],
                        rhs=es[:, hh, :],
                        start=(mi == 0),
                        stop=(mi == MT - 1),
                    )
            # Deferred normalization: y_unnorm = Wo_aug @ U runs straight
            # from a bf16 copy of U (frees the single psu slot fast), and the
            # 1/r scale is applied per column on the y tiles afterwards.
            # The row sum is staged to SBUF via ACT (reciprocal_approx_fast
            # misbehaves on PSUM inputs).
            rsb = rp.tile([1, 2, 512], f32, tag="rsb", name=f"rs{nj}")
            nc.scalar.activation(
                rsb.rearrange("p h n -> p (h n)"),
                u_ps[D:D + 1, :, :].rearrange("p h n -> p (h n)"), Copy)
            u_bf = usp.tile([D + 1, NCH], bf16, tag="usb", name=f"ub{nj}")
            nc.vector.tensor_copy(u_bf, u_ps.rearrange("p h n -> p (h n)"))
            for hh in range(2):
                rin = rp.tile([1, 512], f32, tag="rin", name=f"ri{nj}_{hh}")
                nc.vector.reciprocal_approx_fast(rin, rsb[:, hh, :])
                rb = rbp.tile([128, 512], f32, tag="rb", name=f"rb{nj}_{hh}")
                nc.gpsimd.partition_broadcast(rb, rin)
                yo = yop.tile([128, C1T, 512], f32, tag="yo",
                              name=f"yo{nj}_{hh}")
                for t in range(C1T):
                    y_ps = ps_y.tile([128, 512], f32, tag="psy",
                                     name=f"y{nj}_{hh}_{t}")
                    nc.tensor.matmul(
                        y_ps,
                        lhsT=wo_v[:, t * 128:(t + 1) * 128],
                        rhs=u_bf[:, hh * 512:(hh + 1) * 512],
                        start=True,
                        stop=True,
                    )
                    ys = onp.tile([128, 512], f32, tag="ys",
                                  name=f"ys{nj}_{hh}_{t}")
                    nc.vector.tensor_mul(ys, y_ps, rb)
                    nc.vector.tensor_add(
                        yo[:, t, :],
                        x1t[nj][:, t, hh * 512:(hh + 1) * 512].bitcast(f32),
                        ys,
                    )
                nc.gpsimd.dma_start(
                    out=outv[:, :, nj * NCH + hh * 512:nj * NCH + (hh + 1) * 512],
                    in_=yo)
    nc.compile()
    return nc


def _get_nc():
    if "nc" not in _CACHE:
        _CACHE["nc"] = _build()
    return _CACHE["nc"]


def _prep_in_maps(x1, x2, Wq, bq, Wk, bk, Wv, bv, Wo, bo):
    import ml_dtypes

    bf16 = ml_dtypes.bfloat16
    f32 = np.float32
    x1 = np.asarray(x1, f32)
    x2 = np.asarray(x2, f32)
    Wq = np.asarray(Wq, f32)
    Wk = np.asarray(Wk, f32)
    Wv = np.asarray(Wv, f32)
    Wo = np.asarray(Wo, f32)
    bq = np.asarray(bq, f32)
    bk = np.asarray(bk, f32)
    bv = np.asarray(bv, f32)
    bo = np.asarray(bo, f32)

    # bk is softmax-invariant (constant per score column) and is dropped.
    # bq enters scores via K_aug row 64 = bq^T K (paired with Q ones row).
    # bv folds into the output bias because attention rows sum to one.
    wk = 0.25 * Wk
    wk_aug = np.concatenate([wk, (bq @ wk)[None, :]], axis=0)    # [65, C2]
    wk_t = np.ascontiguousarray(
        wk_aug.T.reshape(C2T, 128, D + 1).transpose(1, 0, 2).reshape(128, -1)
    )  # [128, C2T*65]
    wv_t = np.ascontiguousarray(
        (0.25 * Wv).T.reshape(C2T, 128, D).transpose(1, 0, 2).reshape(128, -1)
    )  # [128, C2T*64]
    bo_eff = bo + Wo @ bv
    wo_aug = np.concatenate([Wo.T, bo_eff[None, :]], axis=0)     # [65, C1]
    wo_pad = np.zeros((128, C1), f32)
    wo_pad[: D + 1] = wo_aug
    cbuf = np.concatenate([wk_t, wv_t, wo_pad], axis=1).astype(bf16)
    assert cbuf.shape == (128, _CB_F)

    wqt = np.ascontiguousarray(
        Wq.T.reshape(C1T, 128, D).transpose(1, 0, 2)
    )  # [128, C1T, D]

    shared = {"cb": cbuf, "wq": wqt}
    in_maps = []
    for b in range(B):
        m = dict(shared)
        m["x1"] = np.ascontiguousarray(x1[b].reshape(C1, HW))
        m["x2"] = np.ascontiguousarray(x2[b].reshape(C2, HW))
        in_maps.append(m)
    return in_maps


def run(inputs, trace=False, **trace_kwargs):
    from concourse.bass_utils import run_bass_kernel_spmd

    nc = _get_nc()
    in_maps = _prep_in_maps(**inputs)
    res = run_bass_kernel_spmd(
        nc, in_maps, list(range(B)), trace=trace, **trace_kwargs
    )
    out = np.stack([res.results[i]["out"] for i in range(B)])
    out = out.reshape(B, C1, H, W).astype(np.float32)
    return out, res


def kernel(**inputs) -> np.ndarray:
    out, _ = run(inputs, trace=False)
    return out


# revision 20
# speedup vs baseline: 1.2690x; 1.0361x over previous
"""Trainium2 Bass kernel for pooled cross-attention block (dense_transformer).

Reference computation per batch element b (B=8, one per NeuronCore):
  x2p = 2x2 mean-pool(x2)                      [512, 32, 32]
  Q = Wq @ x1  + bq                            [64, 4096]   (d-part layout)
  K = Wk @ x2p + bk                            [64, 1024]
  V = Wv @ x2p + bv                            [64, 1024]
  attn = softmax_n(Q^T K)                      [4096, 1024]
  out  = attn @ V^T                            [4096, 64]
  y    = out @ Wo^T + bo -> [256, 4096] ; result = x1 + y

Kernel strategy (all on-chip per core, streamed over n in 1024-col chunks):
  - scores computed TRANSPOSED: sT[m, n] = K^T Q so softmax's reduce dim m
    is the partition dim; the row-sum r[n] comes free from a ones column in
    the augmented V (row 64 of U = V_aug^T expS).
  - bias algebra (exact): bk drops (softmax shift invariance); bq enters via
    an extra K row (K_aug row 64 = bq^T K, paired with a ones row in Q_aug);
    bv folds into bo' = bo + Wo@bv (attn rows sum to 1); bo' enters via the
    ones-row of normalized U against an augmented Wo^T.
  - 2x2 pooling: two strided DVE adds; 1/4 folded into Wk/Wv on host.
  - performance: the PE must stream continuously to hold its warm 2.4 GHz
    clock, so the scores->exp->U loop is pipelined 2 deep (2-bank score
    tiles, 1024-wide exp), all DMAs are >=1MiB and loads are issued up
    front on sync (stores go via gpsimd SWDGE), and the reciprocal uses the
    fast custom-DVE approximation.
"""

import sys

for _p in ("/opt/trn_rl_repo",):
    if _p not in sys.path:
        sys.path.insert(0, _p)

import numpy as np

B, C1, C2, H, W, D = 8, 256, 512, 64, 64, 64
HW = H * W               # n (query) size: 4096
M = (H // 2) * (W // 2)  # kv size: 1024
NCH = 1024               # n-chunk
NCHUNKS = HW // NCH      # 4
C1T = C1 // 128          # 2
C2T = C2 // 128          # 4
MT = M // 128            # 8

# packed bf16 const layout (free-dim offsets)
_WK_OFF = 0              # [128, C2T, 65]
_WV_OFF = C2T * 65       # [128, C2T, 64]
_WO_OFF = _WV_OFF + C2T * 64   # [65->128, 256]
_CB_F = _WO_OFF + C1     # 772

_CACHE = {}


def _build():
    import concourse.bass as bass
    import concourse.tile as tile
    from concourse import bacc, mybir

    dt = mybir.dt
    f32, bf16, f32r = dt.float32, dt.bfloat16, dt.float32r
    Exp = mybir.ActivationFunctionType.Exp

    nc = bacc.Bacc(
        "TRN2", target_bir_lowering=False, debug=False, num_devices=8
    )
    x1 = nc.dram_tensor("x1", [C1, HW], f32r, kind="ExternalInput").ap()
    x2 = nc.dram_tensor("x2", [C2, HW], f32, kind="ExternalInput").ap()
    cb = nc.dram_tensor("cb", [128, _CB_F], bf16, kind="ExternalInput").ap()
    wq = nc.dram_tensor("wq", [128, C1T, D], f32r, kind="ExternalInput").ap()
    out = nc.dram_tensor("out", [C1, HW], f32, kind="ExternalOutput").ap()

    x1v = x1.rearrange("(t p) n -> p t n", t=C1T)
    x2v = x2.rearrange("(c p) n -> p c n", c=C2T)
    outv = out.rearrange("(t p) n -> p t n", t=C1T)

    from contextlib import ExitStack

    with tile.TileContext(nc) as tc, ExitStack() as ctx:
        pool = lambda name, bufs, **kw: ctx.enter_context(
            tc.tile_pool(name=name, bufs=bufs, **kw)
        )
        consts = pool("consts", 1)
        x2p = pool("x2p", 8)
        poolp = pool("poolp", 2)
        x1p = pool("x1p", 4)
        esp = pool("esp", 3)
        rp = pool("rp", 2)
        usp = pool("usp", 2)
        rbp = pool("rbp", 2)
        onp = pool("onp", 3)
        yop = pool("yop", 2)
        ps_s = pool("ps_s", 2, space="PSUM")   # [128,2,512] f32: 2 banks x2
        ps_u = pool("ps_u", 1, space="PSUM")   # [65,2,512] f32: 2 banks
        ps_y = pool("ps_y", 2, space="PSUM")   # [128,512] f32: 1 bank x2

        # ---- constants (2 DMAs) ---------------------------------------
        cb_sb = consts.tile([128, _CB_F], bf16, tag="cb")
        nc.sync.dma_start(out=cb_sb, in_=cb)
        wq_sb = consts.tile([128, C1T, D], f32r, tag="wq")
        nc.sync.dma_start(out=wq_sb, in_=wq)
        wk_v = cb_sb[:, _WK_OFF:_WK_OFF + C2T * 65].rearrange(
            "p (c d) -> p c d", c=C2T)
        wv_v = cb_sb[:, _WV_OFF:_WV_OFF + C2T * 64].rearrange(
            "p (c d) -> p c d", c=C2T)
        wo_v = cb_sb[0:D + 1, _WO_OFF:_WO_OFF + C1]

        # persistent activations
        s_bf = consts.tile([128, C2T, M], bf16, tag="sbf")     # pooled x2
        k_aug = consts.tile([128, M], bf16, tag="kaug")
        nc.gpsimd.memset(k_aug[D:128, :], 0.0)
        v_aug = consts.tile([128, MT, D + 1], bf16, tag="vaug")
        q_aug = consts.tile([128, HW], bf16, tag="qaug")
        nc.gpsimd.memset(q_aug[D:128, :], 1.0)

        # ---- input DMAs (issue order == program order on sync) --------
        x2t = {}
        x1t = [x1p.tile([128, C1T, NCH], f32r, tag="x1t", name=f"x1_{nj}")
               for nj in range(NCHUNKS)]

        def load_x1(nj):
            nc.sync.dma_start(
                out=x1t[nj], in_=x1v[:, :, nj * NCH:(nj + 1) * NCH])

        for h in range(2):
            for ci in range(C2T):
                t = x2p.tile([128, HW // 2], f32, tag="x2t", name=f"x2_{ci}_{h}")
                nc.sync.dma_start(
                    out=t, in_=x2v[:, ci, h * (HW // 2):(h + 1) * (HW // 2)]
                )
                x2t[(ci, h)] = t
            if h == 0:
                load_x1(0)
                load_x1(1)

        # ---- phase A: pool x2, project K_aug and V^T ------------------
        k_ps = ps_u.tile([D + 1, 2, NCH // 2], f32, tag="psu", name="k_ps")
        v_ps = ps_y.tile([128, MT, D], f32, tag="psy", name="v_ps")
        for h in range(2):
            for ci in range(C2T):
                blk = x2t[(ci, h)].rearrange(
                    "p (hh w2 two) -> p hh w2 two", w2=W // 2, two=2)
                t1 = poolp.tile([128, (H // 2) * (W // 2)], f32,
                                tag="t1", name=f"t1_{ci}_{h}")
                t1v = t1.rearrange("p (hh w2) -> p hh w2", w2=W // 2)
                eng1 = nc.gpsimd if (h * C2T + ci) % 2 == 0 else nc.vector
                eng1.tensor_add(t1v, blk[:, :, :, 0], blk[:, :, :, 1])
                t2 = t1.rearrange("p (h2 two w2) -> p h2 two w2",
                                  two=2, w2=W // 2)
                sdst = s_bf[:, ci, h * (M // 2):(h + 1) * (M // 2)].rearrange(
                    "p (h2 w2) -> p h2 w2", w2=W // 2)
                nc.vector.tensor_add(sdst, t2[:, :, 0, :], t2[:, :, 1, :])
                nc.tensor.matmul(
                    k_ps[:, h, :],
                    lhsT=wk_v[:, ci, :],
                    rhs=s_bf[:, ci, h * (M // 2):(h + 1) * (M // 2)],
                    start=(ci == 0),
                    stop=(ci == C2T - 1),
                )
            # V^T accumulation: ci must be the INNER loop — start=True
            # clears has_written for the whole PSUM bank, so a later mi's
            # start would wipe an in-flight mi's accumulation bits.
            for mj in range(MT // 2):
                mi = h * (MT // 2) + mj
                for ci in range(C2T):
                    nc.tensor.matmul(
                        v_ps[:, mi, :],
                        lhsT=s_bf[:, ci, mi * 128:(mi + 1) * 128],
                        rhs=wv_v[:, ci, :],
                        start=(ci == 0),
                        stop=(ci == C2T - 1),
                    )
            # evacuate this half's K and V^T immediately. h0 goes on ACT so
            # the first chunk's exps queue right behind it; h1 goes on DVE
            # to keep the late h1 work from head-of-line-blocking the exps.
            Copy = mybir.ActivationFunctionType.Copy
            if h == 0:
                nc.scalar.activation(
                    k_aug[0:D + 1, 0:M // 2], k_ps[:, 0, :], Copy)
                for mi in range(MT // 2):
                    nc.scalar.activation(v_aug[:, mi, 0:D], v_ps[:, mi, :], Copy)
            else:
                nc.vector.tensor_copy(
                    k_aug[0:D + 1, M // 2:M], k_ps[:, 1, :])
                for mi in range(MT // 2, MT):
                    nc.vector.tensor_copy(v_aug[:, mi, 0:D], v_ps[:, mi, :])
        nc.gpsimd.memset(v_aug[:, :, D], 1.0)
        # dummy reads create a WAR dep so the x1c2/c3 DMA issue is held
        # until K is done - they must not steal HBM bandwidth from x2.
        gate = poolp.tile([1, 2], f32, tag="gate")
        nc.vector.tensor_add(
            gate[:, 0:1], x1t[2][0:1, 0, 0:1].bitcast(f32),
            k_aug[0:1, M - 1:M])
        nc.vector.tensor_add(
            gate[:, 1:2], x1t[3][0:1, 0, 0:1].bitcast(f32),
            k_aug[0:1, M - 1:M])
        load_x1(2)
        load_x1(3)

        # ---- Q projections: nj=0,1 up front, nj+2 prefetched per chunk -
        def make_q(nj):
            q_ps = ps_s.tile([D, 2, NCH // 2], f32, tag="pss", name=f"q_ps{nj}")
            for hh in range(2):
                for ci in range(C1T):
                    nc.tensor.matmul(
                        q_ps[:, hh, :],
                        lhsT=wq_sb[:, ci, :],
                        rhs=x1t[nj][:, ci, hh * 512:(hh + 1) * 512],
                        start=(ci == 0),
                        stop=(ci == C1T - 1),
                    )
            nc.vector.tensor_copy(
                q_aug[0:D, nj * NCH:(nj + 1) * NCH],
                q_ps.rearrange("p h n -> p (h n)"))

        make_q(0)
        make_q(1)

        # ---- phase B: stream n-chunks ---------------------------------
        for nj in range(NCHUNKS):
            u_ps = ps_u.tile([D + 1, 2, 512], f32, tag="psu", name=f"u{nj}")
            for mi in range(MT):
                if mi == 4 and nj + 2 < NCHUNKS:
                    make_q(nj + 2)
                s_t = ps_s.tile([128, 2, 512], f32, tag="pss", name=f"s{nj}_{mi}")
                for hh in range(2):
                    nc.tensor.matmul(
                        s_t[:, hh, :],
                        lhsT=k_aug[:, mi * 128:(mi + 1) * 128],
                        rhs=q_aug[:, nj * NCH + hh * 512:nj * NCH + (hh + 1) * 512],
                        start=True,
                        stop=True,
                    )
                es = esp.tile([128, 2, 512], bf16, tag="es", name=f"es{nj}_{mi}")
                nc.scalar.activation(
                    es.rearrange("p h n -> p (h n)"),
                    s_t.rearrange("p h n -> p (h n)"),
                    Exp,
                )
                for hh in range(2):
                    nc.tensor.matmul(
                        u_ps[:, hh, :],
                        lhsT=v_aug[:, mi, :],
                        rhs=es[:, hh, :],
                        start=(mi == 0),
                        stop=(mi == MT - 1),
                    )
            # Deferred normalization: y_unnorm = Wo_aug @ U runs straight
            # from a bf16 copy of U (frees the single psu slot fast), and the
            # 1/r scale is applied per column on the y tiles afterwards.
            # The row sum is staged to SBUF via ACT (reciprocal_approx_fast
            # misbehaves on PSUM inputs).
            rsb = rp.tile([1, 2, 512], f32, tag="rsb", name=f"rs{nj}")
            nc.scalar.activation(
                rsb.rearrange("p h n -> p (h n)"),
                u_ps[D:D + 1, :, :].rearrange("p h n -> p (h n)"), Copy)
            u_bf = usp.tile([D + 1, NCH], bf16, tag="usb", name=f"ub{nj}")
            nc.vector.tensor_copy(u_bf, u_ps.rearrange("p h n -> p (h n)"))
            for hh in range(2):
                rin = rp.tile([1, 512], f32, tag="rin", name=f"ri{nj}_{hh}")
                nc.vector.reciprocal_approx_fast(rin, rsb[:, hh, :])
                rb = rbp.tile([128, 512], f32, tag="rb", name=f"rb{nj}_{hh}")
                nc.gpsimd.partition_broadcast(rb, rin)
                yo = yop.tile([128, C1T, 512], f32, tag="yo",
                              name=f"yo{nj}_{hh}")
                for t in range(C1T):
                    y_ps = ps_y.tile([128, 512], f32, tag="psy",
                                     name=f"y{nj}_{hh}_{t}")
                    nc.tensor.matmul(
                        y_ps,
                        lhsT=wo_v[:, t * 128:(t + 1) * 128],
                        rhs=u_bf[:, hh * 512:(hh + 1) * 512],
                        start=True,
                        stop=True,
                    )
                    ys = onp.tile([128, 512], f32, tag="ys",
                                  name=f"ys{nj}_{hh}_{t}")
                    nc.vector.tensor_mul(ys, y_ps, rb)
                    nc.vector.tensor_add(
                        yo[:, t, :],
                        x1t[nj][:, t, hh * 512:(hh + 1) * 512].bitcast(f32),
                        ys,
                    )
                nc.gpsimd.dma_start(
                    out=outv[:, :, nj * NCH + hh * 512:nj * NCH + (hh + 1) * 512],
                    in_=yo)
    nc.compile()
    return nc


def _get_nc():
    if "nc" not in _CACHE:
        _CACHE["nc"] = _build()
    return _CACHE["nc"]


def _prep_in_maps(x1, x2, Wq, bq, Wk, bk, Wv, bv, Wo, bo):
    import ml_dtypes

    bf16 = ml_dtypes.bfloat16
    f32 = np.float32
    x1 = np.asarray(x1, f32)
    x2 = np.asarray(x2, f32)
    Wq = np.asarray(Wq, f32)
    Wk = np.asarray(Wk, f32)
    Wv = np.asarray(Wv, f32)
    Wo = np.asarray(Wo, f32)
    bq = np.asarray(bq, f32)
    bk = np.asarray(bk, f32)
    bv = np.asarray(bv, f32)
    bo = np.asarray(bo, f32)

    # bk is softmax-invariant (constant per score column) and is dropped.
    # bq enters scores via K_aug row 64 = bq^T K (paired with Q ones row).
    # bv folds into the output bias because attention rows sum to one.
    wk = 0.25 * Wk
    wk_aug = np.concatenate([wk, (bq @ wk)[None, :]], axis=0)    # [65, C2]
    wk_t = np.ascontiguousarray(
        wk_aug.T.reshape(C2T, 128, D + 1).transpose(1, 0, 2).reshape(128, -1)
    )  # [128, C2T*65]
    wv_t = np.ascontiguousarray(
        (0.25 * Wv).T.reshape(C2T, 128, D).transpose(1, 0, 2).reshape(128, -1)
    )  # [128, C2T*64]
    bo_eff = bo + Wo @ bv
    wo_aug = np.concatenate([Wo.T, bo_eff[None, :]], axis=0)     # [65, C1]
    wo_pad = np.zeros((128, C1), f32)
    wo_pad[: D + 1] = wo_aug
    cbuf = np.concatenate([wk_t, wv_t, wo_pad], axis=1).astype(bf16)
    assert cbuf.shape == (128, _CB_F)

    wqt = np.ascontiguousarray(
        Wq.T.reshape(C1T, 128, D).transpose(1, 0, 2)
    )  # [128, C1T, D]

    shared = {"cb": cbuf, "wq": wqt}
    in_maps = []
    for b in range(B):
        m = dict(shared)
        m["x1"] = np.ascontiguousarray(x1[b].reshape(C1, HW))
        m["x2"] = np.ascontiguousarray(x2[b].reshape(C2, HW))
        in_maps.append(m)
    return in_maps


def run(inputs, trace=False, **trace_kwargs):
    from concourse.bass_utils import run_bass_kernel_spmd

    nc = _get_nc()
    in_maps = _prep_in_maps(**inputs)
    res = run_bass_kernel_spmd(
        nc, in_maps, list(range(B)), trace=trace, **trace_kwargs
    )
    out = np.stack([res.results[i]["out"] for i in range(B)])
    out = out.reshape(B, C1, H, W).astype(np.float32)
    return out, res


def kernel(**inputs) -> np.ndarray:
    out, _ = run(inputs, trace=False)
    return out


# revision 21
# speedup vs baseline: 1.3045x; 1.0280x over previous
"""Trainium2 Bass kernel for pooled cross-attention block (dense_transformer).

Reference computation per batch element b (B=8, one per NeuronCore):
  x2p = 2x2 mean-pool(x2)                      [512, 32, 32]
  Q = Wq @ x1  + bq                            [64, 4096]   (d-part layout)
  K = Wk @ x2p + bk                            [64, 1024]
  V = Wv @ x2p + bv                            [64, 1024]
  attn = softmax_n(Q^T K)                      [4096, 1024]
  out  = attn @ V^T                            [4096, 64]
  y    = out @ Wo^T + bo -> [256, 4096] ; result = x1 + y

Kernel strategy (all on-chip per core, streamed over n in 1024-col chunks):
  - scores computed TRANSPOSED: sT[m, n] = K^T Q so softmax's reduce dim m
    is the partition dim; the row-sum r[n] comes free from a ones column in
    the augmented V (row 64 of U = V_aug^T expS).
  - bias algebra (exact): bk drops (softmax shift invariance); bq enters via
    an extra K row (K_aug row 64 = bq^T K, paired with a ones row in Q_aug);
    bv folds into bo' = bo + Wo@bv (attn rows sum to 1); bo' enters via the
    ones-row of normalized U against an augmented Wo^T.
  - 2x2 pooling: two strided DVE adds; 1/4 folded into Wk/Wv on host.
  - performance: the PE must stream continuously to hold its warm 2.4 GHz
    clock, so the scores->exp->U loop is pipelined 2 deep (2-bank score
    tiles, 1024-wide exp), all DMAs are >=1MiB and loads are issued up
    front on sync (stores go via gpsimd SWDGE), and the reciprocal uses the
    fast custom-DVE approximation.
"""

import sys

for _p in ("/opt/trn_rl_repo",):
    if _p not in sys.path:
        sys.path.insert(0, _p)

import numpy as np

B, C1, C2, H, W, D = 8, 256, 512, 64, 64, 64
HW = H * W               # n (query) size: 4096
M = (H // 2) * (W // 2)  # kv size: 1024
NCH = 1024               # n-chunk
NCHUNKS = HW // NCH      # 4
C1T = C1 // 128          # 2
C2T = C2 // 128          # 4
MT = M // 128            # 8

# packed bf16 const layout (free-dim offsets)
_WK_OFF = 0              # [128, C2T, 65]
_WV_OFF = C2T * 65       # [128, C2T, 64]
_WO_OFF = _WV_OFF + C2T * 64   # [65->128, 256]
_CB_F = _WO_OFF + C1     # 772

_CACHE = {}


def _build():
    import concourse.bass as bass
    import concourse.tile as tile
    from concourse import bacc, mybir

    dt = mybir.dt
    f32, bf16, f32r = dt.float32, dt.bfloat16, dt.float32r
    Exp = mybir.ActivationFunctionType.Exp

    nc = bacc.Bacc(
        "TRN2", target_bir_lowering=False, debug=False, num_devices=8
    )
    x1 = nc.dram_tensor("x1", [C1, HW], f32r, kind="ExternalInput").ap()
    x2 = nc.dram_tensor("x2", [C2, HW], f32, kind="ExternalInput").ap()
    cb = nc.dram_tensor("cb", [128, _CB_F], bf16, kind="ExternalInput").ap()
    wq = nc.dram_tensor("wq", [128, C1T, D], f32r, kind="ExternalInput").ap()
    out = nc.dram_tensor("out", [C1, HW], f32, kind="ExternalOutput").ap()

    x1v = x1.rearrange("(t p) n -> p t n", t=C1T)
    x2v = x2.rearrange("(c p) n -> p c n", c=C2T)
    outv = out.rearrange("(t p) n -> p t n", t=C1T)

    from contextlib import ExitStack

    with tile.TileContext(nc) as tc, ExitStack() as ctx:
        pool = lambda name, bufs, **kw: ctx.enter_context(
            tc.tile_pool(name=name, bufs=bufs, **kw)
        )
        consts = pool("consts", 1)
        x2p = pool("x2p", 8)
        poolp = pool("poolp", 2)
        x1p = pool("x1p", 4)
        esp = pool("esp", 3)
        rp = pool("rp", 2)
        usp = pool("usp", 2)
        rbp = pool("rbp", 2)
        onp = pool("onp", 3)
        yop = pool("yop", 2)
        ps_s = pool("ps_s", 2, space="PSUM")   # [128,2,512] f32: 2 banks x2
        ps_u = pool("ps_u", 1, space="PSUM")   # [65,2,512] f32: 2 banks
        ps_y = pool("ps_y", 2, space="PSUM")   # [128,512] f32: 1 bank x2

        # ---- constants (2 DMAs) ---------------------------------------
        cb_sb = consts.tile([128, _CB_F], bf16, tag="cb")
        nc.sync.dma_start(out=cb_sb, in_=cb)
        wq_sb = consts.tile([128, C1T, D], f32r, tag="wq")
        nc.sync.dma_start(out=wq_sb, in_=wq)
        wk_v = cb_sb[:, _WK_OFF:_WK_OFF + C2T * 65].rearrange(
            "p (c d) -> p c d", c=C2T)
        wv_v = cb_sb[:, _WV_OFF:_WV_OFF + C2T * 64].rearrange(
            "p (c d) -> p c d", c=C2T)
        wo_v = cb_sb[0:D + 1, _WO_OFF:_WO_OFF + C1]

        # persistent activations
        s_bf = consts.tile([128, C2T, M], bf16, tag="sbf")     # pooled x2
        k_aug = consts.tile([128, M], bf16, tag="kaug")
        nc.gpsimd.memset(k_aug[D:128, :], 0.0)
        v_aug = consts.tile([128, MT, D + 1], bf16, tag="vaug")
        q_aug = consts.tile([128, HW], bf16, tag="qaug")
        nc.gpsimd.memset(q_aug[D:128, :], 1.0)

        # ---- input DMAs (issue order == program order on sync) --------
        x2t = {}
        x1t = [x1p.tile([128, C1T, NCH], f32r, tag="x1t", name=f"x1_{nj}")
               for nj in range(NCHUNKS)]

        def load_x1(nj):
            nc.sync.dma_start(
                out=x1t[nj], in_=x1v[:, :, nj * NCH:(nj + 1) * NCH])

        load_x1(0)
        for h in range(2):
            for ci in range(C2T):
                t = x2p.tile([128, HW // 2], f32, tag="x2t", name=f"x2_{ci}_{h}")
                nc.sync.dma_start(
                    out=t, in_=x2v[:, ci, h * (HW // 2):(h + 1) * (HW // 2)]
                )
                x2t[(ci, h)] = t
        load_x1(1)

        # ---- phase A: pool x2, project K_aug and V^T ------------------
        k_ps = ps_u.tile([D + 1, 2, NCH // 2], f32, tag="psu", name="k_ps")
        v_ps = ps_y.tile([128, MT, D], f32, tag="psy", name="v_ps")
        for h in range(2):
            for ci in range(C2T):
                blk = x2t[(ci, h)].rearrange(
                    "p (hh w2 two) -> p hh w2 two", w2=W // 2, two=2)
                t1 = poolp.tile([128, (H // 2) * (W // 2)], f32,
                                tag="t1", name=f"t1_{ci}_{h}")
                t1v = t1.rearrange("p (hh w2) -> p hh w2", w2=W // 2)
                eng1 = nc.gpsimd if (h * C2T + ci) % 2 == 0 else nc.vector
                eng1.tensor_add(t1v, blk[:, :, :, 0], blk[:, :, :, 1])
                t2 = t1.rearrange("p (h2 two w2) -> p h2 two w2",
                                  two=2, w2=W // 2)
                sdst = s_bf[:, ci, h * (M // 2):(h + 1) * (M // 2)].rearrange(
                    "p (h2 w2) -> p h2 w2", w2=W // 2)
                nc.vector.tensor_add(sdst, t2[:, :, 0, :], t2[:, :, 1, :])
                nc.tensor.matmul(
                    k_ps[:, h, :],
                    lhsT=wk_v[:, ci, :],
                    rhs=s_bf[:, ci, h * (M // 2):(h + 1) * (M // 2)],
                    start=(ci == 0),
                    stop=(ci == C2T - 1),
                )
            # V^T accumulation: ci must be the INNER loop — start=True
            # clears has_written for the whole PSUM bank, so a later mi's
            # start would wipe an in-flight mi's accumulation bits.
            for mj in range(MT // 2):
                mi = h * (MT // 2) + mj
                for ci in range(C2T):
                    nc.tensor.matmul(
                        v_ps[:, mi, :],
                        lhsT=s_bf[:, ci, mi * 128:(mi + 1) * 128],
                        rhs=wv_v[:, ci, :],
                        start=(ci == 0),
                        stop=(ci == C2T - 1),
                    )
            # evacuate this half's K and V^T immediately. h0 goes on ACT so
            # the first chunk's exps queue right behind it; h1 goes on DVE
            # to keep the late h1 work from head-of-line-blocking the exps.
            Copy = mybir.ActivationFunctionType.Copy
            if h == 0:
                nc.scalar.activation(
                    k_aug[0:D + 1, 0:M // 2], k_ps[:, 0, :], Copy)
                for mi in range(MT // 2):
                    nc.scalar.activation(v_aug[:, mi, 0:D], v_ps[:, mi, :], Copy)
            else:
                nc.vector.tensor_copy(
                    k_aug[0:D + 1, M // 2:M], k_ps[:, 1, :])
                for mi in range(MT // 2, MT):
                    nc.vector.tensor_copy(v_aug[:, mi, 0:D], v_ps[:, mi, :])
        nc.gpsimd.memset(v_aug[:, :, D], 1.0)
        # dummy reads create a WAR dep so the x1c2/c3 DMA issue is held
        # until K is done - they must not steal HBM bandwidth from x2.
        gate = poolp.tile([1, 2], f32, tag="gate")
        nc.vector.tensor_add(
            gate[:, 0:1], x1t[2][0:1, 0, 0:1].bitcast(f32),
            k_aug[0:1, M - 1:M])
        nc.vector.tensor_add(
            gate[:, 1:2], x1t[3][0:1, 0, 0:1].bitcast(f32),
            k_aug[0:1, M - 1:M])
        load_x1(2)
        load_x1(3)

        # ---- Q projections: nj=0,1 up front, nj+2 prefetched per chunk -
        def make_q(nj):
            q_ps = ps_s.tile([D, 2, NCH // 2], f32, tag="pss", name=f"q_ps{nj}")
            for hh in range(2):
                for ci in range(C1T):
                    nc.tensor.matmul(
                        q_ps[:, hh, :],
                        lhsT=wq_sb[:, ci, :],
                        rhs=x1t[nj][:, ci, hh * 512:(hh + 1) * 512],
                        start=(ci == 0),
                        stop=(ci == C1T - 1),
                    )
            nc.vector.tensor_copy(
                q_aug[0:D, nj * NCH:(nj + 1) * NCH],
                q_ps.rearrange("p h n -> p (h n)"))

        make_q(0)
        make_q(1)

        # ---- phase B: stream n-chunks ---------------------------------
        for nj in range(NCHUNKS):
            u_ps = ps_u.tile([D + 1, 2, 512], f32, tag="psu", name=f"u{nj}")
            for mi in range(MT):
                if mi == 4 and nj + 2 < NCHUNKS:
                    make_q(nj + 2)
                s_t = ps_s.tile([128, 2, 512], f32, tag="pss", name=f"s{nj}_{mi}")
                for hh in range(2):
                    nc.tensor.matmul(
                        s_t[:, hh, :],
                        lhsT=k_aug[:, mi * 128:(mi + 1) * 128],
                        rhs=q_aug[:, nj * NCH + hh * 512:nj * NCH + (hh + 1) * 512],
                        start=True,
                        stop=True,
                    )
                es = esp.tile([128, 2, 512], bf16, tag="es", name=f"es{nj}_{mi}")
                nc.scalar.activation(
                    es.rearrange("p h n -> p (h n)"),
                    s_t.rearrange("p h n -> p (h n)"),
                    Exp,
                )
                for hh in range(2):
                    nc.tensor.matmul(
                        u_ps[:, hh, :],
                        lhsT=v_aug[:, mi, :],
                        rhs=es[:, hh, :],
                        start=(mi == 0),
                        stop=(mi == MT - 1),
                    )
            # Deferred normalization: y_unnorm = Wo_aug @ U runs straight
            # from a bf16 copy of U (frees the single psu slot fast), and the
            # 1/r scale is applied per column on the y tiles afterwards.
            # The row sum is staged to SBUF via ACT (reciprocal_approx_fast
            # misbehaves on PSUM inputs).
            rsb = rp.tile([1, 2, 512], f32, tag="rsb", name=f"rs{nj}")
            nc.scalar.activation(
                rsb.rearrange("p h n -> p (h n)"),
                u_ps[D:D + 1, :, :].rearrange("p h n -> p (h n)"), Copy)
            u_bf = usp.tile([D + 1, NCH], bf16, tag="usb", name=f"ub{nj}")
            nc.vector.tensor_copy(u_bf, u_ps.rearrange("p h n -> p (h n)"))
            for hh in range(2):
                rin = rp.tile([1, 512], f32, tag="rin", name=f"ri{nj}_{hh}")
                nc.vector.reciprocal_approx_fast(rin, rsb[:, hh, :])
                rb = rbp.tile([128, 512], f32, tag="rb", name=f"rb{nj}_{hh}")
                nc.gpsimd.partition_broadcast(rb, rin)
                yo = yop.tile([128, C1T, 512], f32, tag="yo",
                              name=f"yo{nj}_{hh}")
                for t in range(C1T):
                    y_ps = ps_y.tile([128, 512], f32, tag="psy",
                                     name=f"y{nj}_{hh}_{t}")
                    nc.tensor.matmul(
                        y_ps,
                        lhsT=wo_v[:, t * 128:(t + 1) * 128],
                        rhs=u_bf[:, hh * 512:(hh + 1) * 512],
                        start=True,
                        stop=True,
                    )
                    ys = onp.tile([128, 512], f32, tag="ys",
                                  name=f"ys{nj}_{hh}_{t}")
                    nc.vector.tensor_mul(ys, y_ps, rb)
                    nc.vector.tensor_add(
                        yo[:, t, :],
                        x1t[nj][:, t, hh * 512:(hh + 1) * 512].bitcast(f32),
                        ys,
                    )
                nc.gpsimd.dma_start(
                    out=outv[:, :, nj * NCH + hh * 512:nj * NCH + (hh + 1) * 512],
                    in_=yo)
    nc.compile()
    return nc


def _get_nc():
    if "nc" not in _CACHE:
        _CACHE["nc"] = _build()
    return _CACHE["nc"]


def _prep_in_maps(x1, x2, Wq, bq, Wk, bk, Wv, bv, Wo, bo):
    import ml_dtypes

    bf16 = ml_dtypes.bfloat16
    f32 = np.float32
    x1 = np.asarray(x1, f32)
    x2 = np.asarray(x2, f32)
    Wq = np.asarray(Wq, f32)
    Wk = np.asarray(Wk, f32)
    Wv = np.asarray(Wv, f32)
    Wo = np.asarray(Wo, f32)
    bq = np.asarray(bq, f32)
    bk = np.asarray(bk, f32)
    bv = np.asarray(bv, f32)
    bo = np.asarray(bo, f32)

    # bk is softmax-invariant (constant per score column) and is dropped.
    # bq enters scores via K_aug row 64 = bq^T K (paired with Q ones row).
    # bv folds into the output bias because attention rows sum to one.
    wk = 0.25 * Wk
    wk_aug = np.concatenate([wk, (bq @ wk)[None, :]], axis=0)    # [65, C2]
    wk_t = np.ascontiguousarray(
        wk_aug.T.reshape(C2T, 128, D + 1).transpose(1, 0, 2).reshape(128, -1)
    )  # [128, C2T*65]
    wv_t = np.ascontiguousarray(
        (0.25 * Wv).T.reshape(C2T, 128, D).transpose(1, 0, 2).reshape(128, -1)
    )  # [128, C2T*64]
    bo_eff = bo + Wo @ bv
    wo_aug = np.concatenate([Wo.T, bo_eff[None, :]], axis=0)     # [65, C1]
    wo_pad = np.zeros((128, C1), f32)
    wo_pad[: D + 1] = wo_aug
    cbuf = np.concatenate([wk_t, wv_t, wo_pad], axis=1).astype(bf16)
    assert cbuf.shape == (128, _CB_F)

    wqt = np.ascontiguousarray(
        Wq.T.reshape(C1T, 128, D).transpose(1, 0, 2)
    )  # [128, C1T, D]

    shared = {"cb": cbuf, "wq": wqt}
    in_maps = []
    for b in range(B):
        m = dict(shared)
        m["x1"] = np.ascontiguousarray(x1[b].reshape(C1, HW))
        m["x2"] = np.ascontiguousarray(x2[b].reshape(C2, HW))
        in_maps.append(m)
    return in_maps


def run(inputs, trace=False, **trace_kwargs):
    from concourse.bass_utils import run_bass_kernel_spmd

    nc = _get_nc()
    in_maps = _prep_in_maps(**inputs)
    res = run_bass_kernel_spmd(
        nc, in_maps, list(range(B)), trace=trace, **trace_kwargs
    )
    out = np.stack([res.results[i]["out"] for i in range(B)])
    out = out.reshape(B, C1, H, W).astype(np.float32)
    return out, res


def kernel(**inputs) -> np.ndarray:
    out, _ = run(inputs, trace=False)
    return out


# revision 23
# speedup vs baseline: 1.3368x; 1.0248x over previous
"""Trainium2 Bass kernel for pooled cross-attention block (dense_transformer).

Reference computation per batch element b (B=8, one per NeuronCore):
  x2p = 2x2 mean-pool(x2)                      [512, 32, 32]
  Q = Wq @ x1  + bq                            [64, 4096]   (d-part layout)
  K = Wk @ x2p + bk                            [64, 1024]
  V = Wv @ x2p + bv                            [64, 1024]
  attn = softmax_n(Q^T K)                      [4096, 1024]
  out  = attn @ V^T                            [4096, 64]
  y    = out @ Wo^T + bo -> [256, 4096] ; result = x1 + y

Kernel strategy (all on-chip per core, streamed over n in 1024-col chunks):
  - scores computed TRANSPOSED: sT[m, n] = K^T Q so softmax's reduce dim m
    is the partition dim; the row-sum r[n] comes free from a ones column in
    the augmented V (row 64 of U = V_aug^T expS).
  - bias algebra (exact): bk drops (softmax shift invariance); bq enters via
    an extra K row (K_aug row 64 = bq^T K, paired with a ones row in Q_aug);
    bv folds into bo' = bo + Wo@bv (attn rows sum to 1); bo' enters via the
    ones-row of normalized U against an augmented Wo^T.
  - 2x2 pooling: two strided DVE adds; 1/4 folded into Wk/Wv on host.
  - performance: the PE must stream continuously to hold its warm 2.4 GHz
    clock, so the scores->exp->U loop is pipelined 2 deep (2-bank score
    tiles, 1024-wide exp), all DMAs are >=1MiB and loads are issued up
    front on sync (stores go via gpsimd SWDGE), and the reciprocal uses the
    fast custom-DVE approximation.
"""

import sys

for _p in ("/opt/trn_rl_repo",):
    if _p not in sys.path:
        sys.path.insert(0, _p)

import numpy as np

B, C1, C2, H, W, D = 8, 256, 512, 64, 64, 64
HW = H * W               # n (query) size: 4096
M = (H // 2) * (W // 2)  # kv size: 1024
NCH = 1024               # n-chunk
NCHUNKS = HW // NCH      # 4
C1T = C1 // 128          # 2
C2T = C2 // 128          # 4
MT = M // 128            # 8

# packed bf16 const layout (free-dim offsets)
_WK_OFF = 0              # [128, C2T, 65]
_WV_OFF = C2T * 65       # [128, C2T, 64]
_WO_OFF = _WV_OFF + C2T * 64   # [65->128, 256]
_CB_F = _WO_OFF + C1     # 772

_CACHE = {}


def _build():
    import concourse.bass as bass
    import concourse.tile as tile
    from concourse import bacc, mybir

    dt = mybir.dt
    f32, bf16, f32r = dt.float32, dt.bfloat16, dt.float32r
    Exp = mybir.ActivationFunctionType.Exp

    nc = bacc.Bacc(
        "TRN2", target_bir_lowering=False, debug=False, num_devices=8
    )
    x1 = nc.dram_tensor("x1", [C1, HW], f32r, kind="ExternalInput").ap()
    x2 = nc.dram_tensor("x2", [C2, HW], f32, kind="ExternalInput").ap()
    cb = nc.dram_tensor("cb", [128, _CB_F], bf16, kind="ExternalInput").ap()
    wq = nc.dram_tensor("wq", [128, C1T, D], f32r, kind="ExternalInput").ap()
    out = nc.dram_tensor("out", [C1, HW], f32, kind="ExternalOutput").ap()

    x1v = x1.rearrange("(t p) n -> p t n", t=C1T)
    x2v = x2.rearrange("(c p) n -> p c n", c=C2T)
    outv = out.rearrange("(t p) n -> p t n", t=C1T)

    from contextlib import ExitStack

    with tile.TileContext(nc) as tc, ExitStack() as ctx:
        pool = lambda name, bufs, **kw: ctx.enter_context(
            tc.tile_pool(name=name, bufs=bufs, **kw)
        )
        consts = pool("consts", 1)
        x2p = pool("x2p", 8)
        poolp = pool("poolp", 2)
        x1p = pool("x1p", 4)
        esp = pool("esp", 3)
        rp = pool("rp", 2)
        usp = pool("usp", 2)
        rbp = pool("rbp", 2)
        onp = pool("onp", 3)
        yop = pool("yop", 2)
        ps_s = pool("ps_s", 2, space="PSUM")   # [128,2,512] f32: 2 banks x2
        ps_u = pool("ps_u", 1, space="PSUM")   # [65,2,512] f32: 2 banks
        ps_y = pool("ps_y", 2, space="PSUM")   # [128,512] f32: 1 bank x2

        # ---- constants (2 DMAs) ---------------------------------------
        cb_sb = consts.tile([128, _CB_F], bf16, tag="cb")
        nc.sync.dma_start(out=cb_sb, in_=cb)
        wq_sb = consts.tile([128, C1T, D], f32r, tag="wq")
        nc.sync.dma_start(out=wq_sb, in_=wq)
        wk_v = cb_sb[:, _WK_OFF:_WK_OFF + C2T * 65].rearrange(
            "p (c d) -> p c d", c=C2T)
        wv_v = cb_sb[:, _WV_OFF:_WV_OFF + C2T * 64].rearrange(
            "p (c d) -> p c d", c=C2T)
        wo_v = cb_sb[0:D + 1, _WO_OFF:_WO_OFF + C1]

        # persistent activations
        s_bf = consts.tile([128, C2T, M], bf16, tag="sbf")     # pooled x2
        k_aug = consts.tile([128, M], bf16, tag="kaug")
        nc.gpsimd.memset(k_aug[D:128, :], 0.0)
        v_aug = consts.tile([128, MT, D + 1], bf16, tag="vaug")
        nc.gpsimd.memset(v_aug[:, :, D], 1.0)
        q_aug = consts.tile([128, HW], bf16, tag="qaug")
        nc.gpsimd.memset(q_aug[D:128, :], 1.0)

        # ---- input DMAs (issue order == program order on sync) --------
        x2t = {}
        x1t = [x1p.tile([128, C1T, NCH], f32r, tag="x1t", name=f"x1_{nj}")
               for nj in range(NCHUNKS)]

        def load_x1(nj):
            nc.sync.dma_start(
                out=x1t[nj], in_=x1v[:, :, nj * NCH:(nj + 1) * NCH])

        load_x1(0)
        for h in range(2):
            for ci in range(C2T):
                t = x2p.tile([128, HW // 2], f32, tag="x2t", name=f"x2_{ci}_{h}")
                nc.sync.dma_start(
                    out=t, in_=x2v[:, ci, h * (HW // 2):(h + 1) * (HW // 2)]
                )
                x2t[(ci, h)] = t
        load_x1(1)

        # ---- phase A halves + interleaved first chunk ------------------
        Copy = mybir.ActivationFunctionType.Copy
        v_ps = ps_y.tile([128, MT, D], f32, tag="psy", name="v_ps")

        def phase_a(h):
            k_ps = ps_y.tile([D + 1, 512], f32, tag="psy", name=f"k_ps{h}")
            for ci in range(C2T):
                blk = x2t[(ci, h)].rearrange(
                    "p (hh w2 two) -> p hh w2 two", w2=W // 2, two=2)
                t1 = poolp.tile([128, (H // 2) * (W // 2)], f32,
                                tag="t1", name=f"t1_{ci}_{h}")
                t1v = t1.rearrange("p (hh w2) -> p hh w2", w2=W // 2)
                eng1 = nc.gpsimd if (h * C2T + ci) % 2 == 0 else nc.vector
                eng1.tensor_add(t1v, blk[:, :, :, 0], blk[:, :, :, 1])
                t2 = t1.rearrange("p (h2 two w2) -> p h2 two w2",
                                  two=2, w2=W // 2)
                sdst = s_bf[:, ci, h * (M // 2):(h + 1) * (M // 2)].rearrange(
                    "p (h2 w2) -> p h2 w2", w2=W // 2)
                nc.vector.tensor_add(sdst, t2[:, :, 0, :], t2[:, :, 1, :])
                nc.tensor.matmul(
                    k_ps,
                    lhsT=wk_v[:, ci, :],
                    rhs=s_bf[:, ci, h * (M // 2):(h + 1) * (M // 2)],
                    start=(ci == 0),
                    stop=(ci == C2T - 1),
                )
            # V^T accumulation: ci must be the INNER loop — start=True
            # clears has_written for the whole PSUM bank, so a later mi's
            # start would wipe an in-flight mi's accumulation bits.
            for mj in range(MT // 2):
                mi = h * (MT // 2) + mj
                for ci in range(C2T):
                    nc.tensor.matmul(
                        v_ps[:, mi, :],
                        lhsT=s_bf[:, ci, mi * 128:(mi + 1) * 128],
                        rhs=wv_v[:, ci, :],
                        start=(ci == 0),
                        stop=(ci == C2T - 1),
                    )
            # evacuate this half's K and V^T immediately. h0 goes on ACT so
            # the first chunk's exps queue right behind it; h1 goes on DVE
            # to keep the late h1 work from head-of-line-blocking the exps.
            if h == 0:
                nc.scalar.activation(
                    k_aug[0:D + 1, 0:M // 2], k_ps, Copy)
                for mi in range(MT // 2):
                    nc.scalar.activation(v_aug[:, mi, 0:D], v_ps[:, mi, :], Copy)
            else:
                nc.vector.tensor_copy(k_aug[0:D + 1, M // 2:M], k_ps)
                for mi in range(MT // 2, MT):
                    nc.vector.tensor_copy(v_aug[:, mi, 0:D], v_ps[:, mi, :])

        def make_q(nj):
            q_ps = ps_s.tile([D, 2, NCH // 2], f32, tag="pss", name=f"q_ps{nj}")
            for hh in range(2):
                for ci in range(C1T):
                    nc.tensor.matmul(
                        q_ps[:, hh, :],
                        lhsT=wq_sb[:, ci, :],
                        rhs=x1t[nj][:, ci, hh * 512:(hh + 1) * 512],
                        start=(ci == 0),
                        stop=(ci == C1T - 1),
                    )
            nc.vector.tensor_copy(
                q_aug[0:D, nj * NCH:(nj + 1) * NCH],
                q_ps.rearrange("p h n -> p (h n)"))

        def chunk_part(nj, u_ps, mi_lo, mi_hi):
            for mi in range(mi_lo, mi_hi):
                if nj == 1 and mi == 0:
                    make_q(2)
                if nj == 1 and mi == 4:
                    make_q(3)
                s_t = ps_s.tile([128, 2, 512], f32, tag="pss", name=f"s{nj}_{mi}")
                for hh in range(2):
                    nc.tensor.matmul(
                        s_t[:, hh, :],
                        lhsT=k_aug[:, mi * 128:(mi + 1) * 128],
                        rhs=q_aug[:, nj * NCH + hh * 512:nj * NCH + (hh + 1) * 512],
                        start=True,
                        stop=True,
                    )
                es = esp.tile([128, 2, 512], bf16, tag="es", name=f"es{nj}_{mi}")
                nc.scalar.activation(
                    es.rearrange("p h n -> p (h n)"),
                    s_t.rearrange("p h n -> p (h n)"),
                    Exp,
                )
                for hh in range(2):
                    nc.tensor.matmul(
                        u_ps[:, hh, :],
                        lhsT=v_aug[:, mi, :],
                        rhs=es[:, hh, :],
                        start=(mi == 0),
                        stop=(mi == MT - 1),
                    )

        def chunk_tail(nj, u_ps):
            # Deferred normalization: y_unnorm = Wo_aug @ U runs straight
            # from a bf16 copy of U (frees the single psu slot fast), and the
            # 1/r scale is applied per column on the y tiles afterwards.
            # The row sum is staged to SBUF via ACT (reciprocal_approx_fast
            # misbehaves on PSUM inputs).
            rsb = rp.tile([1, 2, 512], f32, tag="rsb", name=f"rs{nj}")
            nc.scalar.activation(
                rsb.rearrange("p h n -> p (h n)"),
                u_ps[D:D + 1, :, :].rearrange("p h n -> p (h n)"), Copy)
            u_bf = usp.tile([D + 1, NCH], bf16, tag="usb", name=f"ub{nj}")
            nc.vector.tensor_copy(u_bf, u_ps.rearrange("p h n -> p (h n)"))
            for hh in range(2):
                rin = rp.tile([1, 512], f32, tag="rin", name=f"ri{nj}_{hh}")
                nc.vector.reciprocal_approx_fast(rin, rsb[:, hh, :])
                rb = rbp.tile([128, 512], f32, tag="rb", name=f"rb{nj}_{hh}")
                nc.gpsimd.partition_broadcast(rb, rin)
                yo = yop.tile([128, C1T, 512], f32, tag="yo",
                              name=f"yo{nj}_{hh}")
                for t in range(C1T):
                    y_ps = ps_y.tile([128, 512], f32, tag="psy",
                                     name=f"y{nj}_{hh}_{t}")
                    nc.tensor.matmul(
                        y_ps,
                        lhsT=wo_v[:, t * 128:(t + 1) * 128],
                        rhs=u_bf[:, hh * 512:(hh + 1) * 512],
                        start=True,
                        stop=True,
                    )
                    ys = onp.tile([128, 512], f32, tag="ys",
                                  name=f"ys{nj}_{hh}_{t}")
                    nc.vector.tensor_mul(ys, y_ps, rb)
                    nc.vector.tensor_add(
                        yo[:, t, :],
                        x1t[nj][:, t, hh * 512:(hh + 1) * 512].bitcast(f32),
                        ys,
                    )
                nc.gpsimd.dma_start(
                    out=outv[:, :, nj * NCH + hh * 512:nj * NCH + (hh + 1) * 512],
                    in_=yo)

        # emission: h0 work, Q0, first half of chunk 0, h1 work, Q1,
        # second half of chunk 0, then chunks 1..3.
        phase_a(0)
        make_q(0)
        u0 = ps_u.tile([D + 1, 2, 512], f32, tag="psu", name="u0")
        chunk_part(0, u0, 0, MT // 2)
        phase_a(1)
        # dummy reads create a WAR dep so the x1c2/c3 DMA issue is held
        # until K is done - they must not steal HBM bandwidth from x2.
        gate = poolp.tile([1, 2], f32, tag="gate")
        nc.vector.tensor_add(
            gate[:, 0:1], x1t[2][0:1, 0, 0:1].bitcast(f32),
            k_aug[0:1, M - 1:M])
        nc.vector.tensor_add(
            gate[:, 1:2], x1t[3][0:1, 0, 0:1].bitcast(f32),
            k_aug[0:1, M - 1:M])
        load_x1(2)
        load_x1(3)
        make_q(1)
        chunk_part(0, u0, MT // 2, MT)
        chunk_tail(0, u0)
        for nj in range(1, NCHUNKS):
            u_ps = ps_u.tile([D + 1, 2, 512], f32, tag="psu", name=f"u{nj}")
            chunk_part(nj, u_ps, 0, MT)
            chunk_tail(nj, u_ps)
    nc.compile()
    return nc


def _get_nc():
    if "nc" not in _CACHE:
        _CACHE["nc"] = _build()
    return _CACHE["nc"]


def _prep_in_maps(x1, x2, Wq, bq, Wk, bk, Wv, bv, Wo, bo):
    import ml_dtypes

    bf16 = ml_dtypes.bfloat16
    f32 = np.float32
    x1 = np.asarray(x1, f32)
    x2 = np.asarray(x2, f32)
    Wq = np.asarray(Wq, f32)
    Wk = np.asarray(Wk, f32)
    Wv = np.asarray(Wv, f32)
    Wo = np.asarray(Wo, f32)
    bq = np.asarray(bq, f32)
    bk = np.asarray(bk, f32)
    bv = np.asarray(bv, f32)
    bo = np.asarray(bo, f32)

    # bk is softmax-invariant (constant per score column) and is dropped.
    # bq enters scores via K_aug row 64 = bq^T K (paired with Q ones row).
    # bv folds into the output bias because attention rows sum to one.
    wk = 0.25 * Wk
    wk_aug = np.concatenate([wk, (bq @ wk)[None, :]], axis=0)    # [65, C2]
    wk_t = np.ascontiguousarray(
        wk_aug.T.reshape(C2T, 128, D + 1).transpose(1, 0, 2).reshape(128, -1)
    )  # [128, C2T*65]
    wv_t = np.ascontiguousarray(
        (0.25 * Wv).T.reshape(C2T, 128, D).transpose(1, 0, 2).reshape(128, -1)
    )  # [128, C2T*64]
    bo_eff = bo + Wo @ bv
    wo_aug = np.concatenate([Wo.T, bo_eff[None, :]], axis=0)     # [65, C1]
    wo_pad = np.zeros((128, C1), f32)
    wo_pad[: D + 1] = wo_aug
    cbuf = np.concatenate([wk_t, wv_t, wo_pad], axis=1).astype(bf16)
    assert cbuf.shape == (128, _CB_F)

    wqt = np.ascontiguousarray(
        Wq.T.reshape(C1T, 128, D).transpose(1, 0, 2)
    )  # [128, C1T, D]

    shared = {"cb": cbuf, "wq": wqt}
    in_maps = []
    for b in range(B):
        m = dict(shared)
        m["x1"] = np.ascontiguousarray(x1[b].reshape(C1, HW))
        m["x2"] = np.ascontiguousarray(x2[b].reshape(C2, HW))
        in_maps.append(m)
    return in_maps


def run(inputs, trace=False, **trace_kwargs):
    from concourse.bass_utils import run_bass_kernel_spmd

    nc = _get_nc()
    in_maps = _prep_in_maps(**inputs)
    res = run_bass_kernel_spmd(
        nc, in_maps, list(range(B)), trace=trace, **trace_kwargs
    )
    out = np.stack([res.results[i]["out"] for i in range(B)])
    out = out.reshape(B, C1, H, W).astype(np.float32)
    return out, res


def kernel(**inputs) -> np.ndarray:
    out, _ = run(inputs, trace=False)
    return out
